# revision 16
# baseline (speedup 1.0000x reference)
"""Distributed 3-layer GAT encoder on 8 TRN2 NeuronCores (Bass/Tile).

Strategy (graph partition by dst, per the sharding hint):
  - Core c owns dst nodes [2500c, 2500c+2500), padded to 2560 = 20 blocks x 128.
  - Layer 1 needs no gather: per-edge source features x[src_e] are a pure
    layout of the *input* x, so they are staged host-side transposed
    (xeT, tiled per dst block); h1|as1 per edge is computed on the PE from
    the streamed xeT tile (one matmul per 128-edge tile).
  - Layers 2-3 gather [h | alpha_src] rows by src from a per-core full
    node table tab_l (DRAM, row pitch 384/128 for 256B-aligned gather)
    via gpsimd dma_gather (descgen-rate-bound at ~8ns/row).
  - Indicator matrices: ind (edges->dst) built on-chip by a DVE broadcast
    is_equal against an iota; indT (dst->edges) is static and streamed
    from a host-precomputed tiled table.
  - alpha_dst expanded per edge via matmul(lhsT=indT, rhs=adloc block);
    p = exp(leaky_relu(as+ad)) computed small [P,Tb,H], then expanded by
    a DVE broadcast multiply (no wide scalar-engine exp).
  - Numerator+denominator accumulated in PSUM via matmuls
    (lhsT=ind, rhs=[p*h | p]).
  - Flush: normalize, mean over heads, bias, relu -> PE transpose ->
    AllGather fp16 (Shared output) -> next layer table rebuild with
    DRAM-contiguous [128,row-pitch] writes.
"""
import numpy as np

N = 20000
NCORES = 8
NPC = 2500
NPAD = 2560
NBLK = 20
NTOT = NCORES * NPAD  # 20480
P = 128

LAST_RESULT = None


# ----------------------------------------------------------------- host prep
def _wrap16(idx, ncols):
    n = len(idx)
    w = np.zeros((P, ncols), dtype=np.int16)
    cols = (n + 15) // 16
    assert cols <= ncols
    buf = np.zeros((16, cols), dtype=np.int16)
    buf[np.arange(n) % 16, np.arange(n) // 16] = idx
    for g in range(8):
        w[16 * g:16 * g + 16, :cols] = buf
    return w


def _preprocess(edge_index, x):
    src = np.asarray(edge_index[0], dtype=np.int64)
    dst = np.asarray(edge_index[1], dtype=np.int64)
    # self-loops are handled locally on-device, not via gather

    own_s = src // NPC
    src_p = own_s * NPAD + (src - own_s * NPC)
    own = dst // NPC
    dst_loc = dst - own * NPC

    order = np.lexsort((dst_loc, own))
    src_p, dst_loc, own = src_p[order], dst_loc[order], own[order]
    blk = dst_loc // P
    counts = np.zeros((NCORES, NBLK), dtype=np.int64)
    for c in range(NCORES):
        for b in range(NBLK):
            counts[c, b] = np.sum((own == c) & (blk == b))
    T = np.maximum(1, np.ceil(counts.max(axis=0) / P).astype(np.int64))
    Ttot = int(T.sum())

    # padded input x (node row n lives at padded id own*NPAD + local)
    x = np.asarray(x, dtype=np.float32)
    xpad = np.zeros((NTOT, 128), dtype=np.float32)
    for c in range(NCORES):
        xpad[c * NPAD:c * NPAD + NPC] = x[c * NPC:(c + 1) * NPC]

    wrap_src = np.zeros((NCORES, P, Ttot * 8), dtype=np.int16)
    dstloc16 = np.full((NCORES, P, Ttot), -1.0, dtype=np.float16)
    indT = np.zeros((NCORES, P, Ttot * P), dtype=np.float16)
    xeT = np.zeros((NCORES, P, Ttot * P), dtype=np.float16)
    off8 = np.zeros(NBLK + 1, dtype=np.int64)
    offT = np.zeros(NBLK + 1, dtype=np.int64)
    for b in range(NBLK):
        off8[b + 1] = off8[b] + T[b] * 8
        offT[b + 1] = offT[b] + T[b]
    prow = np.arange(P, dtype=np.float32)[:, None]
    for c in range(NCORES):
        m_c = own == c
        for b in range(NBLK):
            m = m_c & (blk == b)
            cnt = int(counts[c, b])
            nb = int(T[b]) * P
            isrc = np.zeros(nb, dtype=np.int64)
            isrc[:cnt] = src_p[m]
            dl = np.full(nb, -1.0, dtype=np.float32)
            dl[:cnt] = dst_loc[m] - b * P
            wrap_src[c, :, off8[b]:off8[b + 1]] = _wrap16(isrc, int(T[b]) * 8)
            dstloc16[c, :, offT[b]:offT[b + 1]] = (
                dl.reshape(int(T[b]), P).T.astype(np.float16))
            indT[c, :, offT[b] * P:offT[b + 1] * P] = (
                prow == dl[None, :]).astype(np.float16)
            xeT[c, :, offT[b] * P:offT[b + 1] * P] = (
                xpad[isrc].T.astype(np.float16))
    return T, off8, offT, wrap_src, dstloc16, indT, xeT


# ------------------------------------------------------------- build program
def _build(T, off8, offT, do_compile=True):
    from concourse import bass, bacc, mybir, tile

    f16 = mybir.dt.float16
    f32 = mybir.dt.float32
    i16 = mybir.dt.int16
    AF = mybir.ActivationFunctionType
    OP = mybir.AluOpType

    Ttot = int(T.sum())
    NW = Ttot * 8
    Tmax = int(T.max())
    NVALID_LAST = NPC - (NBLK - 1) * P  # 68

    nc = bacc.Bacc("TRN2", target_bir_lowering=False, debug=False,
                   num_devices=NCORES)

    # inputs
    xlocT = nc.dram_tensor("xlocT", [P, NPAD], f16, kind="ExternalInput")
    iwsrc = nc.dram_tensor("iwsrc", [P, NW], i16, kind="ExternalInput")
    dloc = nc.dram_tensor("dloc", [P, Ttot], f16, kind="ExternalInput")
    indT_d = nc.dram_tensor("indT", [P, Ttot * P], f16, kind="ExternalInput")
    xeT_d = nc.dram_tensor("xeT", [P, Ttot * P], f16, kind="ExternalInput")
    iotabig = nc.dram_tensor("iotabig", [P, Tmax * P], f16,
                             kind="ExternalInput")
    c100 = nc.dram_tensor("c100", [P, 32], f32, kind="ExternalInput")
    c1em8 = nc.dram_tensor("c1em8", [P, 32], f32, kind="ExternalInput")
    ident16 = nc.dram_tensor("ident16", [P, P], f16, kind="ExternalInput")
    identf = nc.dram_tensor("identf", [P, P], f32, kind="ExternalInput")
    w1c = nc.dram_tensor("w1c", [128, 260], f16, kind="ExternalInput")
    w2c = nc.dram_tensor("w2c", [64, 256], f16, kind="ExternalInput")
    asf2 = nc.dram_tensor("asf2", [P, 256], f16, kind="ExternalInput")
    asfr2 = nc.dram_tensor("asfr2", [P, Tmax * 256], f16,
                           kind="ExternalInput")
    w3c = nc.dram_tensor("w3c", [64, 128], f16, kind="ExternalInput")
    wad1 = nc.dram_tensor("wad1", [128, 4], f16, kind="ExternalInput")
    wad2 = nc.dram_tensor("wad2", [64, 4], f16, kind="ExternalInput")
    wad3 = nc.dram_tensor("wad3", [64, 1], f16, kind="ExternalInput")
    b1r = nc.dram_tensor("b1r", [P, 64], f32, kind="ExternalInput")
    b2r = nc.dram_tensor("b2r", [P, 64], f32, kind="ExternalInput")
    b3r = nc.dram_tensor("b3r", [P, 32], f32, kind="ExternalInput")
    bmr = nc.dram_tensor("bmr", [P, 32], f32, kind="ExternalInput")
    bvr = nc.dram_tensor("bvr", [P, 32], f32, kind="ExternalInput")
    wm = nc.dram_tensor("wm", [32, 32], f32, kind="ExternalInput")
    wv = nc.dram_tensor("wv", [32, 32], f32, kind="ExternalInput")

    # outputs
    z_out = nc.dram_tensor("z", [NPC, 32], f32, kind="ExternalOutput")
    zm_out = nc.dram_tensor("zmean", [NPC, 32], f32, kind="ExternalOutput")
    zv_out = nc.dram_tensor("zvar", [NPC, 32], f32, kind="ExternalOutput")

    with tile.TileContext(nc) as tc:
        with (
            tc.tile_pool(name="const", bufs=1) as cpool,
            tc.tile_pool(name="sb", bufs=3) as sb,
            tc.tile_pool(name="gth", bufs=3) as gth,
            tc.tile_pool(name="blk", bufs=3) as blk,
            tc.tile_pool(name="pxp", bufs=3) as pxp,
            tc.tile_pool(name="pswk", bufs=3, space="PSUM") as pswk,
            tc.tile_pool(name="psad", bufs=2, space="PSUM") as psad,
            tc.tile_pool(name="pssm", bufs=1, space="PSUM") as pssm,
            tc.tile_pool(name="psagg", bufs=2, space="PSUM") as psagg,
            tc.tile_pool(name="dram", bufs=1, space="DRAM") as dram,
        ):
            # per-layer node tables: local slice built during the previous
            # layer's flush, then one AllGather -> full Shared table
            tabloc2 = dram.tile([NPAD, 256], f16)
            tabloc3 = dram.tile([NPAD, 128], f16)
            tab2 = dram.tile([NCORES, NPAD, 256], f16,
                             addr_space="Shared")
            tab3 = dram.tile([NCORES, NPAD, 128], f16, addr_space="Shared")
            adloc2 = dram.tile([NPAD, 4], f16)
            adloc3 = dram.tile([NPAD, 1], f16)

            def ld(shape, dt, src):
                t = cpool.tile(shape, dt, tag="c_" + src.name)
                nc.sync.dma_start(out=t[:], in_=src[:, :])
                return t

            id16_sb = ld([P, P], f16, ident16)
            idf_sb = ld([P, P], f32, identf)
            w1c_sb = ld([128, 260], f16, w1c)
            w2c_sb = ld([64, 256], f16, w2c)
            asf2_sb = ld([P, 256], f16, asf2)
            asfr2_sb = ld([P, Tmax * 256], f16, asfr2)
            w3c_sb = ld([64, 128], f16, w3c)
            wad1_sb = ld([128, 4], f16, wad1)
            wad2_sb = ld([64, 4], f16, wad2)
            wad3_sb = ld([64, 1], f16, wad3)
            b1r_sb = ld([P, 64], f32, b1r)
            b2r_sb = ld([P, 64], f32, b2r)
            b3r_sb = ld([P, 32], f32, b3r)
            bmr_sb = ld([P, 32], f32, bmr)
            bvr_sb = ld([P, 32], f32, bvr)
            wm_sb = ld([32, 32], f32, wm)
            wv_sb = ld([32, 32], f32, wv)
            iwsrc_sb = ld([P, NW], i16, iwsrc)
            dloc_sb = ld([P, Ttot], f16, dloc)
            iotabig_sb = ld([P, Tmax * P], f16, iotabig)
            c100_sb = ld([P, 32], f32, c100)
            c1em8_sb = ld([P, 32], f32, c1em8)
            xloc_sb = ld([P, NPAD], f16, xlocT)

            # layer-1 alpha_dst for all blocks, computed once at the head
            adtab1 = cpool.tile([P, NBLK * 4], f16, tag="adtab1")
            for b in range(NBLK):
                pad1 = pssm.tile([P, 4], f32, space="PSUM", tag="sm")
                nc.tensor.matmul(out=pad1[:],
                                 lhsT=xloc_sb[:, b * P:(b + 1) * P],
                                 rhs=wad1_sb[:, :], start=True, stop=True)
                nc.scalar.activation(adtab1[:, b * 4:(b + 1) * 4], pad1[:],
                                     AF.Copy)

            # -------- shared per-block helpers ---------------------------
            def load_indicators(b, Tb):
                # ind[p,t,q] = (dloc[p, offT+t] == q) : edge (t,p) -> dst q
                ind = blk.tile([P, Tb, P], f16, tag="ind")
                nc.vector.tensor_tensor(
                    out=ind[:],
                    in0=dloc_sb[:, int(offT[b]):int(offT[b]) + Tb, None]
                    .to_broadcast([P, Tb, P]),
                    in1=iotabig_sb[:, :Tb * P]
                    .rearrange("p (t q) -> p t q", t=Tb),
                    op=OP.is_equal)
                indT = blk.tile([P, Tb, P], f16, tag="indT")
                nc.sync.dma_start(
                    out=indT[:],
                    in_=indT_d[:, int(offT[b]) * P:int(offT[b + 1]) * P]
                    .rearrange("p (t q) -> p t q", t=Tb))
                return ind, indT

            def leaky_exp_into(pex, pes, Tb, H):
                # pes: [P,Tb,H] f32 pre-activation; writes
                # exp(leaky_relu(pes)) into pex[:, :, HC:HC+H] fp16
                es = sb.tile([P, Tb, H], f32, tag="es")
                nc.vector.tensor_scalar_mul(out=es[:], in0=pes[:],
                                            scalar1=0.2)
                nc.vector.tensor_tensor(out=es[:], in0=es[:], in1=pes[:],
                                        op=OP.max)
                HC = pex.shape[2] - H
                nc.scalar.activation(pex[:, :, HC:HC + H], es[:], AF.Exp)
                return es

            def self_term(as_self, ad_self, h_self, H, C):
                # exp(leaky(as+ad)) * h for the block's own nodes (the
                # self-loop edge), returned as an agg-matmul rhs tile
                HC = H * C
                ess = sb.tile([P, H], f32, tag="ess")
                nc.vector.tensor_tensor(out=ess[:], in0=as_self,
                                        in1=ad_self, op=OP.add)
                es2 = sb.tile([P, H], f32, tag="ess2")
                nc.vector.tensor_scalar_mul(out=es2[:], in0=ess[:],
                                            scalar1=0.2)
                nc.vector.tensor_tensor(out=ess[:], in0=ess[:], in1=es2[:],
                                        op=OP.max)
                pxs = pxp.tile([P, HC + H], f16, tag="pxs")
                nc.scalar.activation(pxs[:, HC:HC + H], ess[:], AF.Exp)
                nc.vector.tensor_tensor(
                    out=pxs[:, 0:HC].rearrange("p (h c) -> p h c", h=H),
                    in0=h_self.rearrange("p (h c) -> p h c", h=H),
                    in1=pxs[:, HC:HC + H, None].to_broadcast([P, H, C]),
                    op=OP.mult)
                return pxs

            # -------- layer 1 stage: streamed per-edge xeT, no gather ----
            def stage1(b):
                H, C = 4, 64
                HC = H * C
                Tb = int(T[b])
                xet = gth.tile([P, Tb * P], f16, tag="xet")
                nc.sync.dma_start(
                    out=xet[:],
                    in_=xeT_d[:, int(offT[b]) * P:int(offT[b + 1]) * P])
                ind, indT = load_indicators(b, Tb)
                adb = adtab1[:, b * 4:(b + 1) * 4]

                # es pre-activation: as1[src_e] + ad1[dst_e]
                pes = psad.tile([P, Tb, H], f32, space="PSUM", tag="pes")
                for t in range(Tb):
                    nc.tensor.matmul(
                        out=pes[:, t, :], lhsT=xet[:, t * P:(t + 1) * P],
                        rhs=w1c_sb[:, HC:HC + H], start=True, stop=False)
                    nc.tensor.matmul(
                        out=pes[:, t, :], lhsT=indT[:, t, :],
                        rhs=adb, start=False, stop=True)
                pex = pxp.tile([P, Tb, HC + H], f16, tag="pex")
                leaky_exp_into(pex, pes, Tb, H)

                # h per edge + p*h, tile by tile
                for t in range(Tb):
                    ph = pswk.tile([P, HC], f32, space="PSUM",
                                   padded_shape=[P, 384], tag="wk")
                    nc.tensor.matmul(
                        out=ph[:], lhsT=xet[:, t * P:(t + 1) * P],
                        rhs=w1c_sb[:, 0:HC], start=True, stop=True)
                    nc.vector.tensor_tensor(
                        out=pex[:, t, 0:HC]
                        .rearrange("p (h c) -> p h c", h=H),
                        in0=ph[:].rearrange("p (h c) -> p h c", h=H),
                        in1=pex[:, t, HC:HC + H, None]
                        .to_broadcast([P, H, C]),
                        op=OP.mult)
                # self-loop term: h1|as1 of the block's own nodes
                phs = pswk.tile([P, HC + H], f32, space="PSUM",
                                padded_shape=[P, 384], tag="wk")
                nc.tensor.matmul(out=phs[:],
                                 lhsT=xloc_sb[:, b * P:(b + 1) * P],
                                 rhs=w1c_sb[:, 0:HC + H],
                                 start=True, stop=True)
                pxs = self_term(phs[:, HC:HC + H], adb, phs[:, 0:HC], H, C)
                return ind, pex, pxs

            # -------- layers 2,3 stage: gather-based ---------------------
            def stage23(tab, adloc, tabloc, elem, H, C, b, asf=None):
                HC = H * C
                Tb = int(T[b])
                nidx = Tb * P
                g = gth.tile([P, Tb, elem], f16, tag="g")
                nc.gpsimd.dma_gather(
                    out_ap=g[:], in_ap=tab[:].rearrange("c n k -> (c n) k"),
                    idxs_ap=iwsrc_sb[:, int(off8[b]):int(off8[b]) + Tb * 8],
                    num_idxs=nidx, num_idxs_reg=nidx, elem_size=elem,
                    elem_step=elem, single_packet=False)
                ind, indT = load_indicators(b, Tb)
                adb = sb.tile([P, H], f16, tag="adb")
                nc.sync.dma_start(out=adb[:],
                                  in_=adloc[b * P:(b + 1) * P, :])

                pad_all = psad.tile([P, Tb, H], f32, space="PSUM", tag="pes")
                for t in range(Tb):
                    nc.tensor.matmul(out=pad_all[:, t, :],
                                     lhsT=indT[:, t, :],
                                     rhs=adb[:], start=True, stop=True)
                pes = sb.tile([P, Tb, H], f32, tag="pess")
                if asf is None:
                    nc.vector.tensor_tensor(out=pes[:],
                                            in0=g[:, :, HC:HC + H],
                                            in1=pad_all[:], op=OP.add)
                else:
                    # alpha_src per edge = sum_c h[e,h,c] * a_s[h,c]
                    tmp = pxp.tile([P, Tb, HC], f16, tag="ast")
                    nc.vector.tensor_tensor(
                        out=tmp[:], in0=g[:, :, 0:HC],
                        in1=asfr2_sb[:, :Tb * HC]
                        .rearrange("p (t k) -> p t k", t=Tb),
                        op=OP.mult)
                    asp = sb.tile([P, Tb, H], f32, tag="asp")
                    nc.vector.tensor_reduce(
                        out=asp[:],
                        in_=tmp[:].rearrange("p t (h c) -> p t h c", h=H),
                        axis=mybir.AxisListType.X, op=OP.add)
                    nc.vector.tensor_tensor(out=pes[:], in0=asp[:],
                                            in1=pad_all[:], op=OP.add)
                tsf = sb.tile([P, elem], f16, tag="tself")
                nc.sync.dma_start(out=tsf[:],
                                  in_=tabloc[b * P:(b + 1) * P, :])
                if asf is None:
                    as_self = tsf[:, HC:HC + H]
                else:
                    tmps = sb.tile([P, HC], f16, tag="tmps")
                    nc.vector.tensor_tensor(out=tmps[:], in0=tsf[:, 0:HC],
                                            in1=asf[:, :HC], op=OP.mult)
                    asps = sb.tile([P, H], f32, tag="asps")
                    nc.vector.tensor_reduce(
                        out=asps[:],
                        in_=tmps[:].rearrange("p (h c) -> p h c", h=H),
                        axis=mybir.AxisListType.X, op=OP.add)
                    as_self = asps[:]
                pxs = self_term(as_self, adb[:], tsf[:, 0:HC], H, C)
                pex = pxp.tile([P, Tb, HC + H], f16, tag="pex")
                es = leaky_exp_into(pex, pes, Tb, H)
                pexf = pxp.tile([P, Tb, HC], f16, tag="pexf")
                nc.scalar.activation(
                    pexf[:].rearrange("p t (h c) -> p t h c", h=H),
                    es[:, :, :, None].to_broadcast([P, Tb, H, C]), AF.Exp)
                nc.vector.tensor_tensor(out=pex[:, :, 0:HC],
                                        in0=g[:, :, 0:HC], in1=pexf[:],
                                        op=OP.mult)
                return ind, pex, pxs

            def agg_flush(b, state, ncols, flush):
                ind, pex, pxs = state
                Tb = int(T[b])
                pa = psagg.tile([P, ncols], f32, space="PSUM", tag="agg")
                for t in range(Tb):
                    nc.tensor.matmul(
                        out=pa[:], lhsT=ind[:, t, :], rhs=pex[:, t, :],
                        start=(t == 0), stop=False)
                nc.tensor.matmul(out=pa[:], lhsT=id16_sb[:],
                                 rhs=pxs[:, :ncols], start=False, stop=True)
                flush(b, pa)

            def edge_layer(stage, ncols, flush):
                # software pipeline: block b's gather/indicator/exp chain is
                # issued before block b-1's aggregation+flush on every engine
                prev = stage(0)
                for b in range(1, NBLK):
                    cur = stage(b)
                    agg_flush(b - 1, prev, ncols, flush)
                    prev = cur
                agg_flush(NBLK - 1, prev, ncols, flush)

            # -------- flush -----------------------------------------------
            def write_adloc(xt_sb_blk, wad_sb, in_c, H, adloc, b):
                pad = pssm.tile([P, 4], f32, space="PSUM", tag="sm")
                nc.tensor.matmul(out=pad[:, :H], lhsT=xt_sb_blk,
                                 rhs=wad_sb[:in_c, :H], start=True, stop=True)
                a16 = sb.tile([P, H], f16, tag="a16")
                nc.scalar.activation(a16[:], pad[:, :H], AF.Copy)
                nc.scalar.dma_start(out=adloc[b * P:(b + 1) * P, :],
                                    in_=a16[:])

            def flush_12(b, pa, brep_sb, wcn_sb, pitch, tabloc, wadn_sb,
                         adlocn, Hn):
                H, C = 4, 64
                HC = H * C
                inv = sb.tile([P, H], f32, tag="inv")
                nc.vector.tensor_scalar_add(out=inv[:], in0=pa[:, HC:HC + H],
                                            scalar1=1e-16)
                nc.vector.reciprocal(out=inv[:], in_=inv[:])
                nc.vector.tensor_scalar_mul(out=inv[:], in0=inv[:],
                                            scalar1=1.0 / H)
                ivx = sb.tile([P, HC], f32, tag="ivx")
                nc.scalar.activation(
                    ivx[:].rearrange("p (h c) -> p h c", h=H),
                    inv[:, :, None].to_broadcast([P, H, C]), AF.Copy)
                nrm = sb.tile([P, HC], f32, tag="nrm")
                nc.vector.tensor_mul(out=nrm[:], in0=pa[:, 0:HC], in1=ivx[:])
                m = sb.tile([P, C], f32, tag="mean")
                nc.vector.tensor_reduce(
                    out=m[:], in_=nrm[:].rearrange("p (h c) -> p c h", h=H),
                    axis=mybir.AxisListType.X, op=OP.add)
                nc.vector.tensor_add(out=m[:], in0=m[:], in1=brep_sb[:, :C])
                x16 = sb.tile([P, C], f16, tag="x16")
                nc.scalar.activation(x16[:], m[:], AF.Relu)
                pt = pssm.tile([C, P], f16, space="PSUM", tag="sm")
                nc.tensor.transpose(out=pt[:], in_=x16[:], identity=id16_sb[:])
                xt = sb.tile([C, P], f16, tag="xt")
                nc.scalar.activation(xt[:], pt[:], AF.Copy)
                # next-layer node-table rows for this core's block
                prt = pswk.tile([P, pitch], f32, space="PSUM",
                                padded_shape=[P, 384], tag="wk")
                nc.tensor.matmul(out=prt[:], lhsT=xt[:],
                                 rhs=wcn_sb[:C, :pitch], start=True, stop=True)
                t16 = sb.tile([P, pitch], f16, tag="trow")
                nc.scalar.activation(t16[:], prt[:], AF.Copy)
                nc.scalar.dma_start(out=tabloc[b * P:(b + 1) * P, :],
                                    in_=t16[:])
                write_adloc(xt[:], wadn_sb, C, Hn, adlocn, b)

            def flush_3(b, pa):
                nvalid = NVALID_LAST if b == NBLK - 1 else P
                inv = sb.tile([P, 1], f32, tag="inv")
                nc.vector.tensor_scalar_add(out=inv[:], in0=pa[:, 32:33],
                                            scalar1=1e-16)
                nc.vector.reciprocal(out=inv[:], in_=inv[:])
                z = sb.tile([P, 32], f32, tag="zf")
                nc.vector.tensor_scalar_mul(out=z[:], in0=pa[:, 0:32],
                                            scalar1=inv[:])
                nc.vector.tensor_add(out=z[:], in0=z[:], in1=b3r_sb[:])
                nc.sync.dma_start(out=z_out[b * P:b * P + nvalid, :],
                                  in_=z[:nvalid, :])
                zt_ps = pssm.tile([32, P], f32, space="PSUM", tag="sm")
                nc.tensor.transpose(out=zt_ps[:], in_=z[:, :32],
                                    identity=idf_sb[:])
                zt = sb.tile([32, P], f32, tag="zt")
                nc.vector.tensor_copy(out=zt[:], in_=zt_ps[:])
                pm = pssm.tile([P, 32], f32, space="PSUM", tag="sm")
                nc.tensor.matmul(out=pm[:], lhsT=zt[:], rhs=wm_sb[:],
                                 start=True, stop=True)
                zm = sb.tile([P, 32], f32, tag="zm")
                nc.vector.tensor_add(out=zm[:], in0=pm[:], in1=bmr_sb[:])
                nc.sync.dma_start(out=zm_out[b * P:b * P + nvalid, :],
                                  in_=zm[:nvalid, :])
                pv = pssm.tile([P, 32], f32, space="PSUM", tag="sm")
                nc.tensor.matmul(out=pv[:], lhsT=zt[:], rhs=wv_sb[:],
                                 start=True, stop=True)
                zv = sb.tile([P, 32], f32, tag="zv")
                nc.vector.tensor_add(out=zv[:], in0=pv[:], in1=bvr_sb[:])
                nc.scalar.activation(zv[:], zv[:], AF.Exp)
                nc.vector.tensor_tensor(out=zv[:], in0=zv[:], in1=c100_sb[:],
                                        op=OP.min)
                nc.vector.tensor_tensor(out=zv[:], in0=zv[:], in1=c1em8_sb[:],
                                        op=OP.max)
                nc.sync.dma_start(out=zv_out[b * P:b * P + nvalid, :],
                                  in_=zv[:nvalid, :])

            # ================ the program ==================================
            edge_layer(stage1, 260,
                       lambda b, pa: flush_12(b, pa, b1r_sb, w2c_sb, 256,
                                              tabloc2, wad2_sb, adloc2, 4))
            nc.gpsimd.collective_compute(
                "AllGather", mybir.AluOpType.bypass,
                replica_groups=[list(range(NCORES))],
                ins=[tabloc2[:]], outs=[tab2[:]])
            edge_layer(lambda b: stage23(tab2, adloc2, tabloc2, 256, 4, 64,
                                         b, asf2_sb), 260,
                       lambda b, pa: flush_12(b, pa, b2r_sb, w3c_sb, 128,
                                              tabloc3, wad3_sb, adloc3, 1))
            nc.gpsimd.collective_compute(
                "AllGather", mybir.AluOpType.bypass,
                replica_groups=[list(range(NCORES))],
                ins=[tabloc3[:]], outs=[tab3[:]])
            edge_layer(lambda b: stage23(tab3, adloc3, tabloc3, 128, 1, 32,
                                         b), 33,
                       flush_3)

    if do_compile:
        nc.compile()
    return nc


def _make_in_maps(x, params, wrap_src, dstloc16, indT, xeT, Tmax):
    x = np.asarray(x, dtype=np.float32)

    def comb(W, a_s, pitch):
        W = np.asarray(W, np.float32)
        a_s = np.asarray(a_s, np.float32)
        heads, c = a_s.shape
        Wr = W.reshape(W.shape[0], heads, c)
        was = np.einsum('ihc,hc->ih', Wr, a_s)
        out = np.zeros((W.shape[0], pitch), dtype=np.float16)
        out[:, :W.shape[1]] = W.astype(np.float16)
        out[:, W.shape[1]:W.shape[1] + heads] = was.astype(np.float16)
        return out

    def wadf(W, a_d):
        W = np.asarray(W, np.float32)
        a_d = np.asarray(a_d, np.float32)
        heads, c = a_d.shape
        Wr = W.reshape(W.shape[0], heads, c)
        return np.einsum('ihc,hc->ih', Wr, a_d).astype(np.float16)

    def rep(v, n=P):
        v = np.asarray(v, np.float32).reshape(1, -1)
        return np.repeat(v, n, axis=0).astype(np.float32)

    common = dict(
        iotabig=np.tile(np.arange(P, dtype=np.float16), (P, Tmax)),
        c100=np.full((P, 32), 100.0, dtype=np.float32),
        c1em8=np.full((P, 32), 1e-8, dtype=np.float32),
        ident16=np.eye(P, dtype=np.float16),
        identf=np.eye(P, dtype=np.float32),
        w1c=comb(params['W1'], params['as1'], 260),
        w2c=np.asarray(params['W2'], np.float32).astype(np.float16),
        asf2=np.tile(np.asarray(params['as2'], np.float32)
                     .reshape(1, -1).astype(np.float16), (P, 1)),
        asfr2=np.tile(np.asarray(params['as2'], np.float32)
                      .reshape(1, -1).astype(np.float16), (P, Tmax)),
        w3c=comb(params['W3'], params['as3'], 128),
        wad1=wadf(params['W1'], params['ad1']),
        wad2=wadf(params['W2'], params['ad2']),
        wad3=wadf(params['W3'], params['ad3']),
        b1r=rep(params['b1']), b2r=rep(params['b2']), b3r=rep(params['b3']),
        bmr=rep(params['bm']), bvr=rep(params['bv']),
        wm=np.asarray(params['Wm'], np.float32),
        wv=np.asarray(params['Wv'], np.float32),
    )
    in_maps = []
    for c in range(NCORES):
        xs = x[c * NPC:(c + 1) * NPC]
        xlocT = np.zeros((P, NPAD), dtype=np.float16)
        xlocT[:, :NPC] = xs.T.astype(np.float16)
        m = dict(common)
        m.update(iwsrc=wrap_src[c], dloc=dstloc16[c], indT=indT[c],
                 xeT=xeT[c], xlocT=xlocT)
        in_maps.append(m)
    return in_maps


# ------------------------------------------------------------------ driver
def kernel(x, edge_index, W1, as1, ad1, b1, W2, as2, ad2, b2,
           W3, as3, ad3, b3, Wm, bm, Wv, bv):
    global LAST_RESULT
    import os
    from concourse.bass_utils import run_bass_kernel_spmd

    T, off8, offT, wrap_src, dstloc16, indT, xeT = _preprocess(
        np.asarray(edge_index), x)
    params = dict(W1=W1, as1=as1, ad1=ad1, b1=b1, W2=W2, as2=as2, ad2=ad2,
                  b2=b2, W3=W3, as3=as3, ad3=ad3, b3=b3, Wm=Wm, bm=bm,
                  Wv=Wv, bv=bv)
    in_maps = _make_in_maps(x, params, wrap_src, dstloc16, indT, xeT,
                            int(T.max()))

    nc = _build(T, off8, offT)
    res = run_bass_kernel_spmd(
        nc, in_maps, core_ids=list(range(NCORES)),
        trace=os.environ.get("BASS_TRACE", "") not in ("", "0"))
    LAST_RESULT = res

    z = np.concatenate([res.results[c]["z"] for c in range(NCORES)], axis=0)
    zm = np.concatenate([res.results[c]["zmean"] for c in range(NCORES)],
                        axis=0)
    zv = np.concatenate([res.results[c]["zvar"] for c in range(NCORES)],
                        axis=0)
    return zm, zv, z


# revision 17
# speedup vs baseline: 1.1177x; 1.1177x over previous
"""Distributed 3-layer GAT encoder on 8 TRN2 NeuronCores (Bass/Tile).

Strategy (graph partition by dst, per the sharding hint):
  - Core c owns dst nodes [2500c, 2500c+2500), padded to 2560 = 20 blocks x 128.
  - Layer 1 needs no gather: per-edge source features x[src_e] are a pure
    layout of the *input* x, so they are staged host-side transposed
    (xeT, tiled per dst block); h1|as1 per edge is computed on the PE from
    the streamed xeT tile (one matmul per 128-edge tile).
  - Layers 2-3 gather [h | alpha_src] rows by src from a per-core full
    node table tab_l (DRAM, row pitch 384/128 for 256B-aligned gather)
    via gpsimd dma_gather (descgen-rate-bound at ~8ns/row).
  - Indicator matrices: ind (edges->dst) built on-chip by a DVE broadcast
    is_equal against an iota; indT (dst->edges) is static and streamed
    from a host-precomputed tiled table.
  - alpha_dst expanded per edge via matmul(lhsT=indT, rhs=adloc block);
    p = exp(leaky_relu(as+ad)) computed small [P,Tb,H], then expanded by
    a DVE broadcast multiply (no wide scalar-engine exp).
  - Numerator+denominator accumulated in PSUM via matmuls
    (lhsT=ind, rhs=[p*h | p]).
  - Flush: normalize, mean over heads, bias, relu -> PE transpose ->
    AllGather fp16 (Shared output) -> next layer table rebuild with
    DRAM-contiguous [128,row-pitch] writes.
"""
import numpy as np

N = 20000
NCORES = 8
NPC = 2500
NPAD = 2560
NBLK = 20
NTOT = NCORES * NPAD  # 20480
P = 128

LAST_RESULT = None


# ----------------------------------------------------------------- host prep
def _wrap16(idx, ncols):
    n = len(idx)
    w = np.zeros((P, ncols), dtype=np.int16)
    cols = (n + 15) // 16
    assert cols <= ncols
    buf = np.zeros((16, cols), dtype=np.int16)
    buf[np.arange(n) % 16, np.arange(n) // 16] = idx
    for g in range(8):
        w[16 * g:16 * g + 16, :cols] = buf
    return w


def _preprocess(edge_index, x):
    src = np.asarray(edge_index[0], dtype=np.int64)
    dst = np.asarray(edge_index[1], dtype=np.int64)
    # self-loops are handled locally on-device, not via gather

    own_s = src // NPC
    src_p = own_s * NPAD + (src - own_s * NPC)
    own = dst // NPC
    dst_loc = dst - own * NPC

    order = np.lexsort((dst_loc, own))
    src_p, dst_loc, own = src_p[order], dst_loc[order], own[order]
    blk = dst_loc // P
    counts = np.zeros((NCORES, NBLK), dtype=np.int64)
    for c in range(NCORES):
        for b in range(NBLK):
            counts[c, b] = np.sum((own == c) & (blk == b))
    T = np.maximum(1, np.ceil(counts.max(axis=0) / P).astype(np.int64))
    Ttot = int(T.sum())

    # padded input x (node row n lives at padded id own*NPAD + local)
    x = np.asarray(x, dtype=np.float32)
    xpad = np.zeros((NTOT, 128), dtype=np.float32)
    for c in range(NCORES):
        xpad[c * NPAD:c * NPAD + NPC] = x[c * NPC:(c + 1) * NPC]

    wrap_src = np.zeros((NCORES, P, Ttot * 8), dtype=np.int16)
    dstloc16 = np.full((NCORES, P, Ttot), -1.0, dtype=np.float16)
    import ml_dtypes
    indT = np.zeros((NCORES, P, Ttot * P), dtype=ml_dtypes.float8_e4m3)
    xeT = np.zeros((NCORES, P, Ttot * P), dtype=np.float16)
    off8 = np.zeros(NBLK + 1, dtype=np.int64)
    offT = np.zeros(NBLK + 1, dtype=np.int64)
    for b in range(NBLK):
        off8[b + 1] = off8[b] + T[b] * 8
        offT[b + 1] = offT[b] + T[b]
    prow = np.arange(P, dtype=np.float32)[:, None]
    for c in range(NCORES):
        m_c = own == c
        for b in range(NBLK):
            m = m_c & (blk == b)
            cnt = int(counts[c, b])
            nb = int(T[b]) * P
            isrc = np.zeros(nb, dtype=np.int64)
            isrc[:cnt] = src_p[m]
            dl = np.full(nb, -1.0, dtype=np.float32)
            dl[:cnt] = dst_loc[m] - b * P
            wrap_src[c, :, off8[b]:off8[b + 1]] = _wrap16(isrc, int(T[b]) * 8)
            dstloc16[c, :, offT[b]:offT[b + 1]] = (
                dl.reshape(int(T[b]), P).T.astype(np.float16))
            indT[c, :, offT[b] * P:offT[b + 1] * P] = (
                prow == dl[None, :]).astype(ml_dtypes.float8_e4m3)
            xeT[c, :, offT[b] * P:offT[b + 1] * P] = (
                xpad[isrc].T.astype(np.float16))
    return T, off8, offT, wrap_src, dstloc16, indT, xeT


# ------------------------------------------------------------- build program
def _build(T, off8, offT, do_compile=True):
    from concourse import bass, bacc, mybir, tile

    f16 = mybir.dt.float16
    f32 = mybir.dt.float32
    i16 = mybir.dt.int16
    f8 = mybir.dt.float8e4
    AF = mybir.ActivationFunctionType
    OP = mybir.AluOpType

    Ttot = int(T.sum())
    NW = Ttot * 8
    Tmax = int(T.max())
    NVALID_LAST = NPC - (NBLK - 1) * P  # 68

    nc = bacc.Bacc("TRN2", target_bir_lowering=False, debug=False,
                   num_devices=NCORES)

    # inputs
    xlocT = nc.dram_tensor("xlocT", [P, NPAD], f16, kind="ExternalInput")
    iwsrc = nc.dram_tensor("iwsrc", [P, NW], i16, kind="ExternalInput")
    dloc = nc.dram_tensor("dloc", [P, Ttot], f16, kind="ExternalInput")
    indT_d = nc.dram_tensor("indT", [P, Ttot * P], f8, kind="ExternalInput")
    xeT_d = nc.dram_tensor("xeT", [P, Ttot * P], f16, kind="ExternalInput")
    iotabig = nc.dram_tensor("iotabig", [P, Tmax * P], f16,
                             kind="ExternalInput")
    c100 = nc.dram_tensor("c100", [P, 32], f32, kind="ExternalInput")
    c1em8 = nc.dram_tensor("c1em8", [P, 32], f32, kind="ExternalInput")
    ident16 = nc.dram_tensor("ident16", [P, P], f16, kind="ExternalInput")
    identf = nc.dram_tensor("identf", [P, P], f32, kind="ExternalInput")
    w1c = nc.dram_tensor("w1c", [128, 260], f16, kind="ExternalInput")
    w2c = nc.dram_tensor("w2c", [64, 256], f16, kind="ExternalInput")
    asf2 = nc.dram_tensor("asf2", [P, 256], f16, kind="ExternalInput")
    asfr2 = nc.dram_tensor("asfr2", [P, Tmax * 256], f16,
                           kind="ExternalInput")
    w3c = nc.dram_tensor("w3c", [64, 128], f16, kind="ExternalInput")
    wad1 = nc.dram_tensor("wad1", [128, 4], f16, kind="ExternalInput")
    wad2 = nc.dram_tensor("wad2", [64, 4], f16, kind="ExternalInput")
    wad3 = nc.dram_tensor("wad3", [64, 1], f16, kind="ExternalInput")
    b1r = nc.dram_tensor("b1r", [P, 64], f32, kind="ExternalInput")
    b2r = nc.dram_tensor("b2r", [P, 64], f32, kind="ExternalInput")
    b3r = nc.dram_tensor("b3r", [P, 32], f32, kind="ExternalInput")
    bmr = nc.dram_tensor("bmr", [P, 32], f32, kind="ExternalInput")
    bvr = nc.dram_tensor("bvr", [P, 32], f32, kind="ExternalInput")
    wm = nc.dram_tensor("wm", [32, 32], f32, kind="ExternalInput")
    wv = nc.dram_tensor("wv", [32, 32], f32, kind="ExternalInput")

    # outputs
    z_out = nc.dram_tensor("z", [NPC, 32], f32, kind="ExternalOutput")
    zm_out = nc.dram_tensor("zmean", [NPC, 32], f32, kind="ExternalOutput")
    zv_out = nc.dram_tensor("zvar", [NPC, 32], f32, kind="ExternalOutput")

    with tile.TileContext(nc) as tc:
        with (
            tc.tile_pool(name="const", bufs=1) as cpool,
            tc.tile_pool(name="sb", bufs=3) as sb,
            tc.tile_pool(name="gth", bufs=3) as gth,
            tc.tile_pool(name="blk", bufs=3) as blk,
            tc.tile_pool(name="pxp", bufs=3) as pxp,
            tc.tile_pool(name="pswk", bufs=3, space="PSUM") as pswk,
            tc.tile_pool(name="psad", bufs=2, space="PSUM") as psad,
            tc.tile_pool(name="pssm", bufs=1, space="PSUM") as pssm,
            tc.tile_pool(name="psagg", bufs=2, space="PSUM") as psagg,
            tc.tile_pool(name="dram", bufs=1, space="DRAM") as dram,
        ):
            # per-layer node tables: local slice built during the previous
            # layer's flush, then one AllGather -> full Shared table
            tabloc2 = dram.tile([NPAD, 256], f16)
            tabloc3 = dram.tile([NPAD, 128], f16)
            tab2 = dram.tile([NCORES, NPAD, 256], f16,
                             addr_space="Shared")
            tab3 = dram.tile([NCORES, NPAD, 128], f16, addr_space="Shared")
            adloc2 = dram.tile([NPAD, 4], f16)
            adloc3 = dram.tile([NPAD, 1], f16)

            def ld(shape, dt, src):
                t = cpool.tile(shape, dt, tag="c_" + src.name)
                nc.sync.dma_start(out=t[:], in_=src[:, :])
                return t

            id16_sb = ld([P, P], f16, ident16)
            idf_sb = ld([P, P], f32, identf)
            w1c_sb = ld([128, 260], f16, w1c)
            w2c_sb = ld([64, 256], f16, w2c)
            asf2_sb = ld([P, 256], f16, asf2)
            asfr2_sb = ld([P, Tmax * 256], f16, asfr2)
            w3c_sb = ld([64, 128], f16, w3c)
            wad1_sb = ld([128, 4], f16, wad1)
            wad2_sb = ld([64, 4], f16, wad2)
            wad3_sb = ld([64, 1], f16, wad3)
            b1r_sb = ld([P, 64], f32, b1r)
            b2r_sb = ld([P, 64], f32, b2r)
            b3r_sb = ld([P, 32], f32, b3r)
            bmr_sb = ld([P, 32], f32, bmr)
            bvr_sb = ld([P, 32], f32, bvr)
            wm_sb = ld([32, 32], f32, wm)
            wv_sb = ld([32, 32], f32, wv)
            iwsrc_sb = ld([P, NW], i16, iwsrc)
            dloc_sb = ld([P, Ttot], f16, dloc)
            iotabig_sb = ld([P, Tmax * P], f16, iotabig)
            c100_sb = ld([P, 32], f32, c100)
            c1em8_sb = ld([P, 32], f32, c1em8)
            xloc_sb = ld([P, NPAD], f16, xlocT)

            # layer-1 alpha_dst for all blocks, computed once at the head
            adtab1 = cpool.tile([P, NBLK * 4], f16, tag="adtab1")
            for b in range(NBLK):
                pad1 = pssm.tile([P, 4], f32, space="PSUM", tag="sm")
                nc.tensor.matmul(out=pad1[:],
                                 lhsT=xloc_sb[:, b * P:(b + 1) * P],
                                 rhs=wad1_sb[:, :], start=True, stop=True)
                nc.scalar.activation(adtab1[:, b * 4:(b + 1) * 4], pad1[:],
                                     AF.Copy)

            # -------- shared per-block helpers ---------------------------
            def load_indicators(b, Tb):
                # ind[p,t,q] = (dloc[p, offT+t] == q) : edge (t,p) -> dst q
                ind = blk.tile([P, Tb, P], f16, tag="ind")
                nc.vector.tensor_tensor(
                    out=ind[:],
                    in0=dloc_sb[:, int(offT[b]):int(offT[b]) + Tb, None]
                    .to_broadcast([P, Tb, P]),
                    in1=iotabig_sb[:, :Tb * P]
                    .rearrange("p (t q) -> p t q", t=Tb),
                    op=OP.is_equal)
                indT = blk.tile([P, Tb, P], f8, tag="indT")
                nc.sync.dma_start(
                    out=indT[:],
                    in_=indT_d[:, int(offT[b]) * P:int(offT[b + 1]) * P]
                    .rearrange("p (t q) -> p t q", t=Tb))
                return ind, indT

            def leaky_exp_into(pex, pes, Tb, H):
                # pes: [P,Tb,H] f32 pre-activation; writes
                # exp(leaky_relu(pes)) into pex[:, :, HC:HC+H] fp16
                es = sb.tile([P, Tb, H], f32, tag="es")
                nc.vector.tensor_scalar_mul(out=es[:], in0=pes[:],
                                            scalar1=0.2)
                nc.vector.tensor_tensor(out=es[:], in0=es[:], in1=pes[:],
                                        op=OP.max)
                HC = pex.shape[2] - H
                nc.scalar.activation(pex[:, :, HC:HC + H], es[:], AF.Exp)
                return es

            def self_term(as_self, ad_self, h_self, H, C):
                # exp(leaky(as+ad)) * h for the block's own nodes (the
                # self-loop edge), returned as an agg-matmul rhs tile
                HC = H * C
                ess = sb.tile([P, H], f32, tag="ess")
                nc.vector.tensor_tensor(out=ess[:], in0=as_self,
                                        in1=ad_self, op=OP.add)
                es2 = sb.tile([P, H], f32, tag="ess2")
                nc.vector.tensor_scalar_mul(out=es2[:], in0=ess[:],
                                            scalar1=0.2)
                nc.vector.tensor_tensor(out=ess[:], in0=ess[:], in1=es2[:],
                                        op=OP.max)
                pxs = pxp.tile([P, HC + H], f16, tag="pxs")
                nc.scalar.activation(pxs[:, HC:HC + H], ess[:], AF.Exp)
                nc.vector.tensor_tensor(
                    out=pxs[:, 0:HC].rearrange("p (h c) -> p h c", h=H),
                    in0=h_self.rearrange("p (h c) -> p h c", h=H),
                    in1=pxs[:, HC:HC + H, None].to_broadcast([P, H, C]),
                    op=OP.mult)
                return pxs

            # -------- layer 1 stage: streamed per-edge xeT, no gather ----
            def stage1(b):
                H, C = 4, 64
                HC = H * C
                Tb = int(T[b])
                xet = gth.tile([P, Tb * P], f16, tag="xet")
                nc.sync.dma_start(
                    out=xet[:],
                    in_=xeT_d[:, int(offT[b]) * P:int(offT[b + 1]) * P])
                ind, indT = load_indicators(b, Tb)
                adb = adtab1[:, b * 4:(b + 1) * 4]

                # es pre-activation: as1[src_e] + ad1[dst_e]
                pes = psad.tile([P, Tb, H], f32, space="PSUM", tag="pes")
                for t in range(Tb):
                    nc.tensor.matmul(
                        out=pes[:, t, :], lhsT=xet[:, t * P:(t + 1) * P],
                        rhs=w1c_sb[:, HC:HC + H], start=True, stop=False)
                    nc.tensor.matmul(
                        out=pes[:, t, :], lhsT=indT[:, t, :],
                        rhs=adb, start=False, stop=True)
                pex = pxp.tile([P, Tb, HC + H], f16, tag="pex")
                leaky_exp_into(pex, pes, Tb, H)

                # h per edge + p*h, tile by tile
                for t in range(Tb):
                    ph = pswk.tile([P, HC], f32, space="PSUM",
                                   padded_shape=[P, 384], tag="wk")
                    nc.tensor.matmul(
                        out=ph[:], lhsT=xet[:, t * P:(t + 1) * P],
                        rhs=w1c_sb[:, 0:HC], start=True, stop=True)
                    nc.vector.tensor_tensor(
                        out=pex[:, t, 0:HC]
                        .rearrange("p (h c) -> p h c", h=H),
                        in0=ph[:].rearrange("p (h c) -> p h c", h=H),
                        in1=pex[:, t, HC:HC + H, None]
                        .to_broadcast([P, H, C]),
                        op=OP.mult)
                # self-loop term: h1|as1 of the block's own nodes
                phs = pswk.tile([P, HC + H], f32, space="PSUM",
                                padded_shape=[P, 384], tag="wk")
                nc.tensor.matmul(out=phs[:],
                                 lhsT=xloc_sb[:, b * P:(b + 1) * P],
                                 rhs=w1c_sb[:, 0:HC + H],
                                 start=True, stop=True)
                pxs = self_term(phs[:, HC:HC + H], adb, phs[:, 0:HC], H, C)
                return ind, pex, pxs

            # -------- layers 2,3 stage: gather-based ---------------------
            def stage23(tab, adloc, tabloc, elem, H, C, b, asf=None):
                HC = H * C
                Tb = int(T[b])
                nidx = Tb * P
                g = gth.tile([P, Tb, elem], f16, tag="g")
                nc.gpsimd.dma_gather(
                    out_ap=g[:], in_ap=tab[:].rearrange("c n k -> (c n) k"),
                    idxs_ap=iwsrc_sb[:, int(off8[b]):int(off8[b]) + Tb * 8],
                    num_idxs=nidx, num_idxs_reg=nidx, elem_size=elem,
                    elem_step=elem, single_packet=False)
                ind, indT = load_indicators(b, Tb)
                adb = sb.tile([P, H], f16, tag="adb")
                nc.sync.dma_start(out=adb[:],
                                  in_=adloc[b * P:(b + 1) * P, :])

                pad_all = psad.tile([P, Tb, H], f32, space="PSUM", tag="pes")
                for t in range(Tb):
                    nc.tensor.matmul(out=pad_all[:, t, :],
                                     lhsT=indT[:, t, :],
                                     rhs=adb[:], start=True, stop=True)
                pes = sb.tile([P, Tb, H], f32, tag="pess")
                if asf is None:
                    nc.vector.tensor_tensor(out=pes[:],
                                            in0=g[:, :, HC:HC + H],
                                            in1=pad_all[:], op=OP.add)
                else:
                    # alpha_src per edge = sum_c h[e,h,c] * a_s[h,c]
                    tmp = pxp.tile([P, Tb, HC], f16, tag="ast")
                    nc.vector.tensor_tensor(
                        out=tmp[:], in0=g[:, :, 0:HC],
                        in1=asfr2_sb[:, :Tb * HC]
                        .rearrange("p (t k) -> p t k", t=Tb),
                        op=OP.mult)
                    asp = sb.tile([P, Tb, H], f32, tag="asp")
                    nc.vector.tensor_reduce(
                        out=asp[:],
                        in_=tmp[:].rearrange("p t (h c) -> p t h c", h=H),
                        axis=mybir.AxisListType.X, op=OP.add)
                    nc.vector.tensor_tensor(out=pes[:], in0=asp[:],
                                            in1=pad_all[:], op=OP.add)
                tsf = sb.tile([P, elem], f16, tag="tself")
                nc.sync.dma_start(out=tsf[:],
                                  in_=tabloc[b * P:(b + 1) * P, :])
                if asf is None:
                    as_self = tsf[:, HC:HC + H]
                else:
                    tmps = sb.tile([P, HC], f16, tag="tmps")
                    nc.vector.tensor_tensor(out=tmps[:], in0=tsf[:, 0:HC],
                                            in1=asf[:, :HC], op=OP.mult)
                    asps = sb.tile([P, H], f32, tag="asps")
                    nc.vector.tensor_reduce(
                        out=asps[:],
                        in_=tmps[:].rearrange("p (h c) -> p h c", h=H),
                        axis=mybir.AxisListType.X, op=OP.add)
                    as_self = asps[:]
                pxs = self_term(as_self, adb[:], tsf[:, 0:HC], H, C)
                pex = pxp.tile([P, Tb, HC + H], f16, tag="pex")
                es = leaky_exp_into(pex, pes, Tb, H)
                pexf = pxp.tile([P, Tb, HC], f16, tag="pexf")
                nc.scalar.activation(
                    pexf[:].rearrange("p t (h c) -> p t h c", h=H),
                    es[:, :, :, None].to_broadcast([P, Tb, H, C]), AF.Exp)
                nc.vector.tensor_tensor(out=pex[:, :, 0:HC],
                                        in0=g[:, :, 0:HC], in1=pexf[:],
                                        op=OP.mult)
                return ind, pex, pxs

            def agg_flush(b, state, ncols, flush):
                ind, pex, pxs = state
                Tb = int(T[b])
                pa = psagg.tile([P, ncols], f32, space="PSUM", tag="agg")
                for t in range(Tb):
                    nc.tensor.matmul(
                        out=pa[:], lhsT=ind[:, t, :], rhs=pex[:, t, :],
                        start=(t == 0), stop=False)
                nc.tensor.matmul(out=pa[:], lhsT=id16_sb[:],
                                 rhs=pxs[:, :ncols], start=False, stop=True)
                flush(b, pa)

            def edge_layer(stage, ncols, flush):
                # software pipeline: block b's gather/indicator/exp chain is
                # issued before block b-1's aggregation+flush on every engine
                prev = stage(0)
                for b in range(1, NBLK):
                    cur = stage(b)
                    agg_flush(b - 1, prev, ncols, flush)
                    prev = cur
                agg_flush(NBLK - 1, prev, ncols, flush)

            # -------- flush -----------------------------------------------
            def write_adloc(xt_sb_blk, wad_sb, in_c, H, adloc, b):
                pad = pssm.tile([P, 4], f32, space="PSUM", tag="sm")
                nc.tensor.matmul(out=pad[:, :H], lhsT=xt_sb_blk,
                                 rhs=wad_sb[:in_c, :H], start=True, stop=True)
                a16 = sb.tile([P, H], f16, tag="a16")
                nc.scalar.activation(a16[:], pad[:, :H], AF.Copy)
                nc.scalar.dma_start(out=adloc[b * P:(b + 1) * P, :],
                                    in_=a16[:])

            def flush_12(b, pa, brep_sb, wcn_sb, pitch, tabloc, wadn_sb,
                         adlocn, Hn):
                H, C = 4, 64
                HC = H * C
                inv = sb.tile([P, H], f32, tag="inv")
                nc.vector.tensor_scalar_add(out=inv[:], in0=pa[:, HC:HC + H],
                                            scalar1=1e-16)
                nc.vector.reciprocal(out=inv[:], in_=inv[:])
                nc.vector.tensor_scalar_mul(out=inv[:], in0=inv[:],
                                            scalar1=1.0 / H)
                ivx = sb.tile([P, HC], f32, tag="ivx")
                nc.scalar.activation(
                    ivx[:].rearrange("p (h c) -> p h c", h=H),
                    inv[:, :, None].to_broadcast([P, H, C]), AF.Copy)
                nrm = sb.tile([P, HC], f32, tag="nrm")
                nc.vector.tensor_mul(out=nrm[:], in0=pa[:, 0:HC], in1=ivx[:])
                m = sb.tile([P, C], f32, tag="mean")
                nc.vector.tensor_reduce(
                    out=m[:], in_=nrm[:].rearrange("p (h c) -> p c h", h=H),
                    axis=mybir.AxisListType.X, op=OP.add)
                nc.vector.tensor_add(out=m[:], in0=m[:], in1=brep_sb[:, :C])
                x16 = sb.tile([P, C], f16, tag="x16")
                nc.scalar.activation(x16[:], m[:], AF.Relu)
                pt = pssm.tile([C, P], f16, space="PSUM", tag="sm")
                nc.tensor.transpose(out=pt[:], in_=x16[:], identity=id16_sb[:])
                xt = sb.tile([C, P], f16, tag="xt")
                nc.scalar.activation(xt[:], pt[:], AF.Copy)
                # next-layer node-table rows for this core's block
                prt = pswk.tile([P, pitch], f32, space="PSUM",
                                padded_shape=[P, 384], tag="wk")
                nc.tensor.matmul(out=prt[:], lhsT=xt[:],
                                 rhs=wcn_sb[:C, :pitch], start=True, stop=True)
                t16 = sb.tile([P, pitch], f16, tag="trow")
                nc.scalar.activation(t16[:], prt[:], AF.Copy)
                nc.scalar.dma_start(out=tabloc[b * P:(b + 1) * P, :],
                                    in_=t16[:])
                write_adloc(xt[:], wadn_sb, C, Hn, adlocn, b)

            def flush_3(b, pa):
                nvalid = NVALID_LAST if b == NBLK - 1 else P
                inv = sb.tile([P, 1], f32, tag="inv")
                nc.vector.tensor_scalar_add(out=inv[:], in0=pa[:, 32:33],
                                            scalar1=1e-16)
                nc.vector.reciprocal(out=inv[:], in_=inv[:])
                z = sb.tile([P, 32], f32, tag="zf")
                nc.vector.tensor_scalar_mul(out=z[:], in0=pa[:, 0:32],
                                            scalar1=inv[:])
                nc.vector.tensor_add(out=z[:], in0=z[:], in1=b3r_sb[:])
                nc.sync.dma_start(out=z_out[b * P:b * P + nvalid, :],
                                  in_=z[:nvalid, :])
                zt_ps = pssm.tile([32, P], f32, space="PSUM", tag="sm")
                nc.tensor.transpose(out=zt_ps[:], in_=z[:, :32],
                                    identity=idf_sb[:])
                zt = sb.tile([32, P], f32, tag="zt")
                nc.vector.tensor_copy(out=zt[:], in_=zt_ps[:])
                pm = pssm.tile([P, 32], f32, space="PSUM", tag="sm")
                nc.tensor.matmul(out=pm[:], lhsT=zt[:], rhs=wm_sb[:],
                                 start=True, stop=True)
                zm = sb.tile([P, 32], f32, tag="zm")
                nc.vector.tensor_add(out=zm[:], in0=pm[:], in1=bmr_sb[:])
                nc.sync.dma_start(out=zm_out[b * P:b * P + nvalid, :],
                                  in_=zm[:nvalid, :])
                pv = pssm.tile([P, 32], f32, space="PSUM", tag="sm")
                nc.tensor.matmul(out=pv[:], lhsT=zt[:], rhs=wv_sb[:],
                                 start=True, stop=True)
                zv = sb.tile([P, 32], f32, tag="zv")
                nc.vector.tensor_add(out=zv[:], in0=pv[:], in1=bvr_sb[:])
                nc.scalar.activation(zv[:], zv[:], AF.Exp)
                nc.vector.tensor_tensor(out=zv[:], in0=zv[:], in1=c100_sb[:],
                                        op=OP.min)
                nc.vector.tensor_tensor(out=zv[:], in0=zv[:], in1=c1em8_sb[:],
                                        op=OP.max)
                nc.sync.dma_start(out=zv_out[b * P:b * P + nvalid, :],
                                  in_=zv[:nvalid, :])

            # ================ the program ==================================
            edge_layer(stage1, 260,
                       lambda b, pa: flush_12(b, pa, b1r_sb, w2c_sb, 256,
                                              tabloc2, wad2_sb, adloc2, 4))
            nc.gpsimd.collective_compute(
                "AllGather", mybir.AluOpType.bypass,
                replica_groups=[list(range(NCORES))],
                ins=[tabloc2[:]], outs=[tab2[:]])
            edge_layer(lambda b: stage23(tab2, adloc2, tabloc2, 256, 4, 64,
                                         b, asf2_sb), 260,
                       lambda b, pa: flush_12(b, pa, b2r_sb, w3c_sb, 128,
                                              tabloc3, wad3_sb, adloc3, 1))
            nc.gpsimd.collective_compute(
                "AllGather", mybir.AluOpType.bypass,
                replica_groups=[list(range(NCORES))],
                ins=[tabloc3[:]], outs=[tab3[:]])
            edge_layer(lambda b: stage23(tab3, adloc3, tabloc3, 128, 1, 32,
                                         b), 33,
                       flush_3)

    if do_compile:
        nc.compile()
    return nc


def _make_in_maps(x, params, wrap_src, dstloc16, indT, xeT, Tmax):
    x = np.asarray(x, dtype=np.float32)

    def comb(W, a_s, pitch):
        W = np.asarray(W, np.float32)
        a_s = np.asarray(a_s, np.float32)
        heads, c = a_s.shape
        Wr = W.reshape(W.shape[0], heads, c)
        was = np.einsum('ihc,hc->ih', Wr, a_s)
        out = np.zeros((W.shape[0], pitch), dtype=np.float16)
        out[:, :W.shape[1]] = W.astype(np.float16)
        out[:, W.shape[1]:W.shape[1] + heads] = was.astype(np.float16)
        return out

    def wadf(W, a_d):
        W = np.asarray(W, np.float32)
        a_d = np.asarray(a_d, np.float32)
        heads, c = a_d.shape
        Wr = W.reshape(W.shape[0], heads, c)
        return np.einsum('ihc,hc->ih', Wr, a_d).astype(np.float16)

    def rep(v, n=P):
        v = np.asarray(v, np.float32).reshape(1, -1)
        return np.repeat(v, n, axis=0).astype(np.float32)

    common = dict(
        iotabig=np.tile(np.arange(P, dtype=np.float16), (P, Tmax)),
        c100=np.full((P, 32), 100.0, dtype=np.float32),
        c1em8=np.full((P, 32), 1e-8, dtype=np.float32),
        ident16=np.eye(P, dtype=np.float16),
        identf=np.eye(P, dtype=np.float32),
        w1c=comb(params['W1'], params['as1'], 260),
        w2c=np.asarray(params['W2'], np.float32).astype(np.float16),
        asf2=np.tile(np.asarray(params['as2'], np.float32)
                     .reshape(1, -1).astype(np.float16), (P, 1)),
        asfr2=np.tile(np.asarray(params['as2'], np.float32)
                      .reshape(1, -1).astype(np.float16), (P, Tmax)),
        w3c=comb(params['W3'], params['as3'], 128),
        wad1=wadf(params['W1'], params['ad1']),
        wad2=wadf(params['W2'], params['ad2']),
        wad3=wadf(params['W3'], params['ad3']),
        b1r=rep(params['b1']), b2r=rep(params['b2']), b3r=rep(params['b3']),
        bmr=rep(params['bm']), bvr=rep(params['bv']),
        wm=np.asarray(params['Wm'], np.float32),
        wv=np.asarray(params['Wv'], np.float32),
    )
    in_maps = []
    for c in range(NCORES):
        xs = x[c * NPC:(c + 1) * NPC]
        xlocT = np.zeros((P, NPAD), dtype=np.float16)
        xlocT[:, :NPC] = xs.T.astype(np.float16)
        m = dict(common)
        m.update(iwsrc=wrap_src[c], dloc=dstloc16[c], indT=indT[c],
                 xeT=xeT[c], xlocT=xlocT)
        in_maps.append(m)
    return in_maps


# ------------------------------------------------------------------ driver
def kernel(x, edge_index, W1, as1, ad1, b1, W2, as2, ad2, b2,
           W3, as3, ad3, b3, Wm, bm, Wv, bv):
    global LAST_RESULT
    import os
    from concourse.bass_utils import run_bass_kernel_spmd

    T, off8, offT, wrap_src, dstloc16, indT, xeT = _preprocess(
        np.asarray(edge_index), x)
    params = dict(W1=W1, as1=as1, ad1=ad1, b1=b1, W2=W2, as2=as2, ad2=ad2,
                  b2=b2, W3=W3, as3=as3, ad3=ad3, b3=b3, Wm=Wm, bm=bm,
                  Wv=Wv, bv=bv)
    in_maps = _make_in_maps(x, params, wrap_src, dstloc16, indT, xeT,
                            int(T.max()))

    nc = _build(T, off8, offT)
    res = run_bass_kernel_spmd(
        nc, in_maps, core_ids=list(range(NCORES)),
        trace=os.environ.get("BASS_TRACE", "") not in ("", "0"))
    LAST_RESULT = res

    z = np.concatenate([res.results[c]["z"] for c in range(NCORES)], axis=0)
    zm = np.concatenate([res.results[c]["zmean"] for c in range(NCORES)],
                        axis=0)
    zv = np.concatenate([res.results[c]["zvar"] for c in range(NCORES)],
                        axis=0)
    return zm, zv, z


# revision 18
# speedup vs baseline: 1.1592x; 1.0371x over previous
"""Distributed 3-layer GAT encoder on 8 TRN2 NeuronCores (Bass/Tile).

Strategy (graph partition by dst, per the sharding hint):
  - Core c owns dst nodes [2500c, 2500c+2500), padded to 2560 = 20 blocks x 128.
  - Layer 1 needs no gather: per-edge source features x[src_e] are a pure
    layout of the *input* x, so they are staged host-side transposed
    (xeT, tiled per dst block); h1|as1 per edge is computed on the PE from
    the streamed xeT tile (one matmul per 128-edge tile).
  - Layers 2-3 gather [h | alpha_src] rows by src from a per-core full
    node table tab_l (DRAM, row pitch 384/128 for 256B-aligned gather)
    via gpsimd dma_gather (descgen-rate-bound at ~8ns/row).
  - Indicator matrices: ind (edges->dst) built on-chip by a DVE broadcast
    is_equal against an iota; indT (dst->edges) is static and streamed
    from a host-precomputed tiled table.
  - alpha_dst expanded per edge via matmul(lhsT=indT, rhs=adloc block);
    p = exp(leaky_relu(as+ad)) computed small [P,Tb,H], then expanded by
    a DVE broadcast multiply (no wide scalar-engine exp).
  - Numerator+denominator accumulated in PSUM via matmuls
    (lhsT=ind, rhs=[p*h | p]).
  - Flush: normalize, mean over heads, bias, relu -> PE transpose ->
    AllGather fp16 (Shared output) -> next layer table rebuild with
    DRAM-contiguous [128,row-pitch] writes.
"""
import numpy as np

N = 20000
NCORES = 8
NPC = 2500
NPAD = 2560
NBLK = 20
NTOT = NCORES * NPAD  # 20480
P = 128

LAST_RESULT = None


# ----------------------------------------------------------------- host prep
def _wrap16(idx, ncols):
    n = len(idx)
    w = np.zeros((P, ncols), dtype=np.int16)
    cols = (n + 15) // 16
    assert cols <= ncols
    buf = np.zeros((16, cols), dtype=np.int16)
    buf[np.arange(n) % 16, np.arange(n) // 16] = idx
    for g in range(8):
        w[16 * g:16 * g + 16, :cols] = buf
    return w


def _preprocess(edge_index, x):
    src = np.asarray(edge_index[0], dtype=np.int64)
    dst = np.asarray(edge_index[1], dtype=np.int64)
    # self-loops are handled locally on-device, not via gather

    own_s = src // NPC
    src_p = own_s * NPAD + (src - own_s * NPC)
    own = dst // NPC
    dst_loc = dst - own * NPC

    order = np.lexsort((dst_loc, own))
    src_p, dst_loc, own = src_p[order], dst_loc[order], own[order]
    blk = dst_loc // P
    counts = np.zeros((NCORES, NBLK), dtype=np.int64)
    for c in range(NCORES):
        for b in range(NBLK):
            counts[c, b] = np.sum((own == c) & (blk == b))
    T = np.maximum(1, np.ceil(counts.max(axis=0) / P).astype(np.int64))
    Ttot = int(T.sum())

    # padded input x (node row n lives at padded id own*NPAD + local)
    x = np.asarray(x, dtype=np.float32)
    xpad = np.zeros((NTOT, 128), dtype=np.float32)
    for c in range(NCORES):
        xpad[c * NPAD:c * NPAD + NPC] = x[c * NPC:(c + 1) * NPC]

    wrap_src = np.zeros((NCORES, P, Ttot * 8), dtype=np.int16)
    dstloc16 = np.full((NCORES, P, Ttot), -1.0, dtype=np.float16)
    import ml_dtypes
    indT = np.zeros((NCORES, P, Ttot * P), dtype=ml_dtypes.float8_e4m3)
    xeT = np.zeros((NCORES, P, Ttot * P), dtype=np.float16)
    off8 = np.zeros(NBLK + 1, dtype=np.int64)
    offT = np.zeros(NBLK + 1, dtype=np.int64)
    for b in range(NBLK):
        off8[b + 1] = off8[b] + T[b] * 8
        offT[b + 1] = offT[b] + T[b]
    prow = np.arange(P, dtype=np.float32)[:, None]
    for c in range(NCORES):
        m_c = own == c
        for b in range(NBLK):
            m = m_c & (blk == b)
            cnt = int(counts[c, b])
            nb = int(T[b]) * P
            isrc = np.zeros(nb, dtype=np.int64)
            isrc[:cnt] = src_p[m]
            dl = np.full(nb, -1.0, dtype=np.float32)
            dl[:cnt] = dst_loc[m] - b * P
            wrap_src[c, :, off8[b]:off8[b + 1]] = _wrap16(isrc, int(T[b]) * 8)
            dstloc16[c, :, offT[b]:offT[b + 1]] = (
                dl.reshape(int(T[b]), P).T.astype(np.float16))
            indT[c, :, offT[b] * P:offT[b + 1] * P] = (
                prow == dl[None, :]).astype(ml_dtypes.float8_e4m3)
            xeT[c, :, offT[b] * P:offT[b + 1] * P] = (
                xpad[isrc].T.astype(np.float16))
    return T, off8, offT, wrap_src, dstloc16, indT, xeT


# ------------------------------------------------------------- build program
def _build(T, off8, offT, do_compile=True):
    from concourse import bass, bacc, mybir, tile

    f16 = mybir.dt.float16
    f32 = mybir.dt.float32
    i16 = mybir.dt.int16
    f8 = mybir.dt.float8e4
    AF = mybir.ActivationFunctionType
    OP = mybir.AluOpType

    Ttot = int(T.sum())
    NW = Ttot * 8
    Tmax = int(T.max())
    NVALID_LAST = NPC - (NBLK - 1) * P  # 68

    nc = bacc.Bacc("TRN2", target_bir_lowering=False, debug=False,
                   num_devices=NCORES)

    # inputs
    xlocT = nc.dram_tensor("xlocT", [P, NPAD], f16, kind="ExternalInput")
    iwsrc = nc.dram_tensor("iwsrc", [P, NW], i16, kind="ExternalInput")
    dloc = nc.dram_tensor("dloc", [P, Ttot], f16, kind="ExternalInput")
    indT_d = nc.dram_tensor("indT", [P, Ttot * P], f8, kind="ExternalInput")
    xeT_d = nc.dram_tensor("xeT", [P, Ttot * P], f16, kind="ExternalInput")
    iotabig = nc.dram_tensor("iotabig", [P, Tmax * P], f16,
                             kind="ExternalInput")
    c100 = nc.dram_tensor("c100", [P, 32], f32, kind="ExternalInput")
    c1em8 = nc.dram_tensor("c1em8", [P, 32], f32, kind="ExternalInput")
    ident16 = nc.dram_tensor("ident16", [P, P], f16, kind="ExternalInput")
    identf = nc.dram_tensor("identf", [P, P], f32, kind="ExternalInput")
    w1c = nc.dram_tensor("w1c", [128, 260], f16, kind="ExternalInput")
    w2c = nc.dram_tensor("w2c", [64, 256], f16, kind="ExternalInput")
    asf2 = nc.dram_tensor("asf2", [P, 256], f16, kind="ExternalInput")
    asfr2 = nc.dram_tensor("asfr2", [P, Tmax * 256], f16,
                           kind="ExternalInput")
    w3c = nc.dram_tensor("w3c", [64, 128], f16, kind="ExternalInput")
    wad1 = nc.dram_tensor("wad1", [128, 4], f16, kind="ExternalInput")
    wad2 = nc.dram_tensor("wad2", [64, 4], f16, kind="ExternalInput")
    wad3 = nc.dram_tensor("wad3", [64, 1], f16, kind="ExternalInput")
    b1r = nc.dram_tensor("b1r", [P, 64], f32, kind="ExternalInput")
    b2r = nc.dram_tensor("b2r", [P, 64], f32, kind="ExternalInput")
    b3r = nc.dram_tensor("b3r", [P, 32], f32, kind="ExternalInput")
    bmr = nc.dram_tensor("bmr", [P, 32], f32, kind="ExternalInput")
    bvr = nc.dram_tensor("bvr", [P, 32], f32, kind="ExternalInput")
    wm = nc.dram_tensor("wm", [32, 32], f32, kind="ExternalInput")
    wv = nc.dram_tensor("wv", [32, 32], f32, kind="ExternalInput")

    # outputs
    z_out = nc.dram_tensor("z", [NPC, 32], f32, kind="ExternalOutput")
    zm_out = nc.dram_tensor("zmean", [NPC, 32], f32, kind="ExternalOutput")
    zv_out = nc.dram_tensor("zvar", [NPC, 32], f32, kind="ExternalOutput")

    with tile.TileContext(nc) as tc:
        with (
            tc.tile_pool(name="const", bufs=1) as cpool,
            tc.tile_pool(name="sb", bufs=3) as sb,
            tc.tile_pool(name="gth", bufs=4) as gth,
            tc.tile_pool(name="blk", bufs=3) as blk,
            tc.tile_pool(name="pxp", bufs=3) as pxp,
            tc.tile_pool(name="pswk", bufs=3, space="PSUM") as pswk,
            tc.tile_pool(name="psad", bufs=2, space="PSUM") as psad,
            tc.tile_pool(name="pssm", bufs=1, space="PSUM") as pssm,
            tc.tile_pool(name="psagg", bufs=2, space="PSUM") as psagg,
            tc.tile_pool(name="dram", bufs=1, space="DRAM") as dram,
        ):
            # per-layer node tables: local slice built during the previous
            # layer's flush, then one AllGather -> full Shared table
            tabloc2 = dram.tile([NPAD, 256], f16)
            tabloc3 = dram.tile([NPAD, 128], f16)
            tab2 = dram.tile([NCORES, NPAD, 256], f16,
                             addr_space="Shared")
            tab3 = dram.tile([NCORES, NPAD, 128], f16, addr_space="Shared")
            adloc2 = dram.tile([NPAD, 4], f16)
            adloc3 = dram.tile([NPAD, 1], f16)

            def ld(shape, dt, src):
                t = cpool.tile(shape, dt, tag="c_" + src.name)
                nc.sync.dma_start(out=t[:], in_=src[:, :])
                return t

            id16_sb = ld([P, P], f16, ident16)
            idf_sb = ld([P, P], f32, identf)
            w1c_sb = ld([128, 260], f16, w1c)
            w2c_sb = ld([64, 256], f16, w2c)
            asf2_sb = ld([P, 256], f16, asf2)
            asfr2_sb = ld([P, Tmax * 256], f16, asfr2)
            w3c_sb = ld([64, 128], f16, w3c)
            wad1_sb = ld([128, 4], f16, wad1)
            wad2_sb = ld([64, 4], f16, wad2)
            wad3_sb = ld([64, 1], f16, wad3)
            b1r_sb = ld([P, 64], f32, b1r)
            b2r_sb = ld([P, 64], f32, b2r)
            b3r_sb = ld([P, 32], f32, b3r)
            bmr_sb = ld([P, 32], f32, bmr)
            bvr_sb = ld([P, 32], f32, bvr)
            wm_sb = ld([32, 32], f32, wm)
            wv_sb = ld([32, 32], f32, wv)
            iwsrc_sb = ld([P, NW], i16, iwsrc)
            dloc_sb = ld([P, Ttot], f16, dloc)
            iotabig_sb = ld([P, Tmax * P], f16, iotabig)
            c100_sb = ld([P, 32], f32, c100)
            c1em8_sb = ld([P, 32], f32, c1em8)
            xloc_sb = ld([P, NPAD], f16, xlocT)

            # layer-1 alpha_dst for all blocks, computed once at the head
            adtab1 = cpool.tile([P, NBLK * 4], f16, tag="adtab1")
            for b in range(NBLK):
                pad1 = pssm.tile([P, 4], f32, space="PSUM", tag="sm")
                nc.tensor.matmul(out=pad1[:],
                                 lhsT=xloc_sb[:, b * P:(b + 1) * P],
                                 rhs=wad1_sb[:, :], start=True, stop=True)
                nc.scalar.activation(adtab1[:, b * 4:(b + 1) * 4], pad1[:],
                                     AF.Copy)

            # -------- shared per-block helpers ---------------------------
            def load_indicators(b, Tb):
                # ind[p,t,q] = (dloc[p, offT+t] == q) : edge (t,p) -> dst q
                ind = blk.tile([P, Tb, P], f16, tag="ind")
                nc.vector.tensor_tensor(
                    out=ind[:],
                    in0=dloc_sb[:, int(offT[b]):int(offT[b]) + Tb, None]
                    .to_broadcast([P, Tb, P]),
                    in1=iotabig_sb[:, :Tb * P]
                    .rearrange("p (t q) -> p t q", t=Tb),
                    op=OP.is_equal)
                indT = blk.tile([P, Tb, P], f8, tag="indT")
                nc.sync.dma_start(
                    out=indT[:],
                    in_=indT_d[:, int(offT[b]) * P:int(offT[b + 1]) * P]
                    .rearrange("p (t q) -> p t q", t=Tb))
                return ind, indT

            def leaky_exp_into(pex, pes, Tb, H):
                # pes: [P,Tb,H] f32 pre-activation; writes
                # exp(leaky_relu(pes)) into pex[:, :, HC:HC+H] fp16
                es = sb.tile([P, Tb, H], f32, tag="es")
                nc.vector.tensor_scalar_mul(out=es[:], in0=pes[:],
                                            scalar1=0.2)
                nc.vector.tensor_tensor(out=es[:], in0=es[:], in1=pes[:],
                                        op=OP.max)
                HC = pex.shape[2] - H
                nc.scalar.activation(pex[:, :, HC:HC + H], es[:], AF.Exp)
                return es

            def self_term(as_self, ad_self, h_self, H, C):
                # exp(leaky(as+ad)) * h for the block's own nodes (the
                # self-loop edge), returned as an agg-matmul rhs tile
                HC = H * C
                ess = sb.tile([P, H], f32, tag="ess")
                nc.vector.tensor_tensor(out=ess[:], in0=as_self,
                                        in1=ad_self, op=OP.add)
                es2 = sb.tile([P, H], f32, tag="ess2")
                nc.vector.tensor_scalar_mul(out=es2[:], in0=ess[:],
                                            scalar1=0.2)
                nc.vector.tensor_tensor(out=ess[:], in0=ess[:], in1=es2[:],
                                        op=OP.max)
                pxs = pxp.tile([P, HC + H], f16, tag="pxs")
                nc.scalar.activation(pxs[:, HC:HC + H], ess[:], AF.Exp)
                nc.vector.tensor_tensor(
                    out=pxs[:, 0:HC].rearrange("p (h c) -> p h c", h=H),
                    in0=h_self.rearrange("p (h c) -> p h c", h=H),
                    in1=pxs[:, HC:HC + H, None].to_broadcast([P, H, C]),
                    op=OP.mult)
                return pxs

            # -------- layer 1 stage: streamed per-edge xeT, no gather ----
            def stage1(b):
                H, C = 4, 64
                HC = H * C
                Tb = int(T[b])
                xet = gth.tile([P, Tb * P], f16, tag="xet")
                nc.sync.dma_start(
                    out=xet[:],
                    in_=xeT_d[:, int(offT[b]) * P:int(offT[b + 1]) * P])
                ind, indT = load_indicators(b, Tb)
                adb = adtab1[:, b * 4:(b + 1) * 4]

                # es pre-activation: as1[src_e] + ad1[dst_e]
                pes = psad.tile([P, Tb, H], f32, space="PSUM", tag="pes")
                for t in range(Tb):
                    nc.tensor.matmul(
                        out=pes[:, t, :], lhsT=xet[:, t * P:(t + 1) * P],
                        rhs=w1c_sb[:, HC:HC + H], start=True, stop=False)
                    nc.tensor.matmul(
                        out=pes[:, t, :], lhsT=indT[:, t, :],
                        rhs=adb, start=False, stop=True)
                pex = pxp.tile([P, Tb, HC + H], f16, tag="pex")
                leaky_exp_into(pex, pes, Tb, H)

                # h per edge + p*h, tile by tile
                for t in range(Tb):
                    ph = pswk.tile([P, HC], f32, space="PSUM",
                                   padded_shape=[P, 384], tag="wk")
                    nc.tensor.matmul(
                        out=ph[:], lhsT=xet[:, t * P:(t + 1) * P],
                        rhs=w1c_sb[:, 0:HC], start=True, stop=True)
                    nc.vector.tensor_tensor(
                        out=pex[:, t, 0:HC]
                        .rearrange("p (h c) -> p h c", h=H),
                        in0=ph[:].rearrange("p (h c) -> p h c", h=H),
                        in1=pex[:, t, HC:HC + H, None]
                        .to_broadcast([P, H, C]),
                        op=OP.mult)
                # self-loop term: h1|as1 of the block's own nodes
                phs = pswk.tile([P, HC + H], f32, space="PSUM",
                                padded_shape=[P, 384], tag="wk")
                nc.tensor.matmul(out=phs[:],
                                 lhsT=xloc_sb[:, b * P:(b + 1) * P],
                                 rhs=w1c_sb[:, 0:HC + H],
                                 start=True, stop=True)
                pxs = self_term(phs[:, HC:HC + H], adb, phs[:, 0:HC], H, C)
                return ind, pex, pxs

            # -------- layers 2,3 stage: gather-based ---------------------
            def stage23(tab, adloc, tabloc, elem, H, C, b, asf=None):
                HC = H * C
                Tb = int(T[b])
                nidx = Tb * P
                g = gth.tile([P, Tb, elem], f16, tag="g")
                nc.gpsimd.dma_gather(
                    out_ap=g[:], in_ap=tab[:].rearrange("c n k -> (c n) k"),
                    idxs_ap=iwsrc_sb[:, int(off8[b]):int(off8[b]) + Tb * 8],
                    num_idxs=nidx, num_idxs_reg=nidx, elem_size=elem,
                    elem_step=elem, single_packet=False)
                ind, indT = load_indicators(b, Tb)
                adb = sb.tile([P, H], f16, tag="adb")
                nc.sync.dma_start(out=adb[:],
                                  in_=adloc[b * P:(b + 1) * P, :])

                pad_all = psad.tile([P, Tb, H], f32, space="PSUM", tag="pes")
                for t in range(Tb):
                    nc.tensor.matmul(out=pad_all[:, t, :],
                                     lhsT=indT[:, t, :],
                                     rhs=adb[:], start=True, stop=True)
                pes = sb.tile([P, Tb, H], f32, tag="pess")
                if asf is None:
                    nc.vector.tensor_tensor(out=pes[:],
                                            in0=g[:, :, HC:HC + H],
                                            in1=pad_all[:], op=OP.add)
                else:
                    # alpha_src per edge = sum_c h[e,h,c] * a_s[h,c]
                    tmp = pxp.tile([P, Tb, HC], f16, tag="ast")
                    nc.vector.tensor_tensor(
                        out=tmp[:], in0=g[:, :, 0:HC],
                        in1=asfr2_sb[:, :Tb * HC]
                        .rearrange("p (t k) -> p t k", t=Tb),
                        op=OP.mult)
                    asp = sb.tile([P, Tb, H], f32, tag="asp")
                    nc.vector.tensor_reduce(
                        out=asp[:],
                        in_=tmp[:].rearrange("p t (h c) -> p t h c", h=H),
                        axis=mybir.AxisListType.X, op=OP.add)
                    nc.vector.tensor_tensor(out=pes[:], in0=asp[:],
                                            in1=pad_all[:], op=OP.add)
                tsf = sb.tile([P, elem], f16, tag="tself")
                nc.sync.dma_start(out=tsf[:],
                                  in_=tabloc[b * P:(b + 1) * P, :])
                if asf is None:
                    as_self = tsf[:, HC:HC + H]
                else:
                    tmps = sb.tile([P, HC], f16, tag="tmps")
                    nc.vector.tensor_tensor(out=tmps[:], in0=tsf[:, 0:HC],
                                            in1=asf[:, :HC], op=OP.mult)
                    asps = sb.tile([P, H], f32, tag="asps")
                    nc.vector.tensor_reduce(
                        out=asps[:],
                        in_=tmps[:].rearrange("p (h c) -> p h c", h=H),
                        axis=mybir.AxisListType.X, op=OP.add)
                    as_self = asps[:]
                pxs = self_term(as_self, adb[:], tsf[:, 0:HC], H, C)
                pex = pxp.tile([P, Tb, HC + H], f16, tag="pex")
                leaky_exp_into(pex, pes, Tb, H)
                nc.vector.tensor_tensor(
                    out=pex[:, :, 0:HC]
                    .rearrange("p t (h c) -> p t h c", h=H),
                    in0=g[:, :, 0:HC]
                    .rearrange("p t (h c) -> p t h c", h=H),
                    in1=pex[:, :, HC:HC + H, None]
                    .to_broadcast([P, Tb, H, C]),
                    op=OP.mult)
                return ind, pex, pxs

            def agg_flush(b, state, ncols, flush):
                ind, pex, pxs = state
                Tb = int(T[b])
                pa = psagg.tile([P, ncols], f32, space="PSUM", tag="agg")
                for t in range(Tb):
                    nc.tensor.matmul(
                        out=pa[:], lhsT=ind[:, t, :], rhs=pex[:, t, :],
                        start=(t == 0), stop=False)
                nc.tensor.matmul(out=pa[:], lhsT=id16_sb[:],
                                 rhs=pxs[:, :ncols], start=False, stop=True)
                flush(b, pa)

            def edge_layer(stage, ncols, flush):
                # software pipeline: block b's gather/indicator/exp chain is
                # issued before block b-1's aggregation+flush on every engine
                prev = stage(0)
                for b in range(1, NBLK):
                    cur = stage(b)
                    agg_flush(b - 1, prev, ncols, flush)
                    prev = cur
                agg_flush(NBLK - 1, prev, ncols, flush)

            # -------- flush -----------------------------------------------
            def write_adloc(xt_sb_blk, wad_sb, in_c, H, adloc, b):
                pad = pssm.tile([P, 4], f32, space="PSUM", tag="sm")
                nc.tensor.matmul(out=pad[:, :H], lhsT=xt_sb_blk,
                                 rhs=wad_sb[:in_c, :H], start=True, stop=True)
                a16 = sb.tile([P, H], f16, tag="a16")
                nc.scalar.activation(a16[:], pad[:, :H], AF.Copy)
                nc.scalar.dma_start(out=adloc[b * P:(b + 1) * P, :],
                                    in_=a16[:])

            def flush_12(b, pa, brep_sb, wcn_sb, pitch, tabloc, wadn_sb,
                         adlocn, Hn):
                H, C = 4, 64
                HC = H * C
                inv = sb.tile([P, H], f32, tag="inv")
                nc.vector.tensor_scalar_add(out=inv[:], in0=pa[:, HC:HC + H],
                                            scalar1=1e-16)
                nc.vector.reciprocal(out=inv[:], in_=inv[:])
                nc.vector.tensor_scalar_mul(out=inv[:], in0=inv[:],
                                            scalar1=1.0 / H)
                nrm = sb.tile([P, HC], f32, tag="nrm")
                nc.vector.tensor_tensor(
                    out=nrm[:].rearrange("p (h c) -> p h c", h=H),
                    in0=pa[:, 0:HC].rearrange("p (h c) -> p h c", h=H),
                    in1=inv[:, :, None].to_broadcast([P, H, C]),
                    op=OP.mult)
                m = sb.tile([P, C], f32, tag="mean")
                nc.vector.tensor_reduce(
                    out=m[:], in_=nrm[:].rearrange("p (h c) -> p c h", h=H),
                    axis=mybir.AxisListType.X, op=OP.add)
                nc.vector.tensor_add(out=m[:], in0=m[:], in1=brep_sb[:, :C])
                x16 = sb.tile([P, C], f16, tag="x16")
                nc.vector.tensor_scalar(out=x16[:], in0=m[:], scalar1=0.0,
                                        scalar2=None, op0=OP.max)
                pt = pssm.tile([C, P], f16, space="PSUM", tag="sm")
                nc.tensor.transpose(out=pt[:], in_=x16[:], identity=id16_sb[:])
                xt = sb.tile([C, P], f16, tag="xt")
                nc.scalar.activation(xt[:], pt[:], AF.Copy)
                # next-layer node-table rows for this core's block
                prt = pswk.tile([P, pitch], f32, space="PSUM",
                                padded_shape=[P, 384], tag="wk")
                nc.tensor.matmul(out=prt[:], lhsT=xt[:],
                                 rhs=wcn_sb[:C, :pitch], start=True, stop=True)
                t16 = sb.tile([P, pitch], f16, tag="trow")
                nc.scalar.activation(t16[:], prt[:], AF.Copy)
                nc.scalar.dma_start(out=tabloc[b * P:(b + 1) * P, :],
                                    in_=t16[:])
                write_adloc(xt[:], wadn_sb, C, Hn, adlocn, b)

            def flush_3(b, pa):
                nvalid = NVALID_LAST if b == NBLK - 1 else P
                inv = sb.tile([P, 1], f32, tag="inv")
                nc.vector.tensor_scalar_add(out=inv[:], in0=pa[:, 32:33],
                                            scalar1=1e-16)
                nc.vector.reciprocal(out=inv[:], in_=inv[:])
                z = sb.tile([P, 32], f32, tag="zf")
                nc.vector.tensor_scalar_mul(out=z[:], in0=pa[:, 0:32],
                                            scalar1=inv[:])
                nc.vector.tensor_add(out=z[:], in0=z[:], in1=b3r_sb[:])
                nc.sync.dma_start(out=z_out[b * P:b * P + nvalid, :],
                                  in_=z[:nvalid, :])
                zt_ps = pssm.tile([32, P], f32, space="PSUM", tag="sm")
                nc.tensor.transpose(out=zt_ps[:], in_=z[:, :32],
                                    identity=idf_sb[:])
                zt = sb.tile([32, P], f32, tag="zt")
                nc.vector.tensor_copy(out=zt[:], in_=zt_ps[:])
                pm = pssm.tile([P, 32], f32, space="PSUM", tag="sm")
                nc.tensor.matmul(out=pm[:], lhsT=zt[:], rhs=wm_sb[:],
                                 start=True, stop=True)
                zm = sb.tile([P, 32], f32, tag="zm")
                nc.vector.tensor_add(out=zm[:], in0=pm[:], in1=bmr_sb[:])
                nc.sync.dma_start(out=zm_out[b * P:b * P + nvalid, :],
                                  in_=zm[:nvalid, :])
                pv = pssm.tile([P, 32], f32, space="PSUM", tag="sm")
                nc.tensor.matmul(out=pv[:], lhsT=zt[:], rhs=wv_sb[:],
                                 start=True, stop=True)
                zv = sb.tile([P, 32], f32, tag="zv")
                nc.vector.tensor_add(out=zv[:], in0=pv[:], in1=bvr_sb[:])
                nc.scalar.activation(zv[:], zv[:], AF.Exp)
                nc.vector.tensor_tensor(out=zv[:], in0=zv[:], in1=c100_sb[:],
                                        op=OP.min)
                nc.vector.tensor_tensor(out=zv[:], in0=zv[:], in1=c1em8_sb[:],
                                        op=OP.max)
                nc.sync.dma_start(out=zv_out[b * P:b * P + nvalid, :],
                                  in_=zv[:nvalid, :])

            # ================ the program ==================================
            edge_layer(stage1, 260,
                       lambda b, pa: flush_12(b, pa, b1r_sb, w2c_sb, 256,
                                              tabloc2, wad2_sb, adloc2, 4))
            nc.gpsimd.collective_compute(
                "AllGather", mybir.AluOpType.bypass,
                replica_groups=[list(range(NCORES))],
                ins=[tabloc2[:]], outs=[tab2[:]])
            edge_layer(lambda b: stage23(tab2, adloc2, tabloc2, 256, 4, 64,
                                         b, asf2_sb), 260,
                       lambda b, pa: flush_12(b, pa, b2r_sb, w3c_sb, 128,
                                              tabloc3, wad3_sb, adloc3, 1))
            nc.gpsimd.collective_compute(
                "AllGather", mybir.AluOpType.bypass,
                replica_groups=[list(range(NCORES))],
                ins=[tabloc3[:]], outs=[tab3[:]])
            edge_layer(lambda b: stage23(tab3, adloc3, tabloc3, 128, 1, 32,
                                         b), 33,
                       flush_3)

    if do_compile:
        nc.compile()
    return nc


def _make_in_maps(x, params, wrap_src, dstloc16, indT, xeT, Tmax):
    x = np.asarray(x, dtype=np.float32)

    def comb(W, a_s, pitch):
        W = np.asarray(W, np.float32)
        a_s = np.asarray(a_s, np.float32)
        heads, c = a_s.shape
        Wr = W.reshape(W.shape[0], heads, c)
        was = np.einsum('ihc,hc->ih', Wr, a_s)
        out = np.zeros((W.shape[0], pitch), dtype=np.float16)
        out[:, :W.shape[1]] = W.astype(np.float16)
        out[:, W.shape[1]:W.shape[1] + heads] = was.astype(np.float16)
        return out

    def wadf(W, a_d):
        W = np.asarray(W, np.float32)
        a_d = np.asarray(a_d, np.float32)
        heads, c = a_d.shape
        Wr = W.reshape(W.shape[0], heads, c)
        return np.einsum('ihc,hc->ih', Wr, a_d).astype(np.float16)

    def rep(v, n=P):
        v = np.asarray(v, np.float32).reshape(1, -1)
        return np.repeat(v, n, axis=0).astype(np.float32)

    common = dict(
        iotabig=np.tile(np.arange(P, dtype=np.float16), (P, Tmax)),
        c100=np.full((P, 32), 100.0, dtype=np.float32),
        c1em8=np.full((P, 32), 1e-8, dtype=np.float32),
        ident16=np.eye(P, dtype=np.float16),
        identf=np.eye(P, dtype=np.float32),
        w1c=comb(params['W1'], params['as1'], 260),
        w2c=np.asarray(params['W2'], np.float32).astype(np.float16),
        asf2=np.tile(np.asarray(params['as2'], np.float32)
                     .reshape(1, -1).astype(np.float16), (P, 1)),
        asfr2=np.tile(np.asarray(params['as2'], np.float32)
                      .reshape(1, -1).astype(np.float16), (P, Tmax)),
        w3c=comb(params['W3'], params['as3'], 128),
        wad1=wadf(params['W1'], params['ad1']),
        wad2=wadf(params['W2'], params['ad2']),
        wad3=wadf(params['W3'], params['ad3']),
        b1r=rep(params['b1']), b2r=rep(params['b2']), b3r=rep(params['b3']),
        bmr=rep(params['bm']), bvr=rep(params['bv']),
        wm=np.asarray(params['Wm'], np.float32),
        wv=np.asarray(params['Wv'], np.float32),
    )
    in_maps = []
    for c in range(NCORES):
        xs = x[c * NPC:(c + 1) * NPC]
        xlocT = np.zeros((P, NPAD), dtype=np.float16)
        xlocT[:, :NPC] = xs.T.astype(np.float16)
        m = dict(common)
        m.update(iwsrc=wrap_src[c], dloc=dstloc16[c], indT=indT[c],
                 xeT=xeT[c], xlocT=xlocT)
        in_maps.append(m)
    return in_maps


# ------------------------------------------------------------------ driver
def kernel(x, edge_index, W1, as1, ad1, b1, W2, as2, ad2, b2,
           W3, as3, ad3, b3, Wm, bm, Wv, bv):
    global LAST_RESULT
    import os
    from concourse.bass_utils import run_bass_kernel_spmd

    T, off8, offT, wrap_src, dstloc16, indT, xeT = _preprocess(
        np.asarray(edge_index), x)
    params = dict(W1=W1, as1=as1, ad1=ad1, b1=b1, W2=W2, as2=as2, ad2=ad2,
                  b2=b2, W3=W3, as3=as3, ad3=ad3, b3=b3, Wm=Wm, bm=bm,
                  Wv=Wv, bv=bv)
    in_maps = _make_in_maps(x, params, wrap_src, dstloc16, indT, xeT,
                            int(T.max()))

    nc = _build(T, off8, offT)
    res = run_bass_kernel_spmd(
        nc, in_maps, core_ids=list(range(NCORES)),
        trace=os.environ.get("BASS_TRACE", "") not in ("", "0"))
    LAST_RESULT = res

    z = np.concatenate([res.results[c]["z"] for c in range(NCORES)], axis=0)
    zm = np.concatenate([res.results[c]["zmean"] for c in range(NCORES)],
                        axis=0)
    zv = np.concatenate([res.results[c]["zvar"] for c in range(NCORES)],
                        axis=0)
    return zm, zv, z


# revision 20
# speedup vs baseline: 1.1625x; 1.0029x over previous
"""Distributed 3-layer GAT encoder on 8 TRN2 NeuronCores (Bass/Tile).

Strategy (graph partition by dst, per the sharding hint):
  - Core c owns dst nodes [2500c, 2500c+2500), padded to 2560 = 20 blocks x 128.
  - Layer 1 needs no gather: per-edge source features x[src_e] are a pure
    layout of the *input* x, so they are staged host-side transposed
    (xeT, tiled per dst block); h1|as1 per edge is computed on the PE from
    the streamed xeT tile (one matmul per 128-edge tile).
  - Layers 2-3 gather [h | alpha_src] rows by src from a per-core full
    node table tab_l (DRAM, row pitch 384/128 for 256B-aligned gather)
    via gpsimd dma_gather (descgen-rate-bound at ~8ns/row).
  - Indicator matrices: ind (edges->dst) built on-chip by a DVE broadcast
    is_equal against an iota; indT (dst->edges) is static and streamed
    from a host-precomputed tiled table.
  - alpha_dst expanded per edge via matmul(lhsT=indT, rhs=adloc block);
    p = exp(leaky_relu(as+ad)) computed small [P,Tb,H], then expanded by
    a DVE broadcast multiply (no wide scalar-engine exp).
  - Numerator+denominator accumulated in PSUM via matmuls
    (lhsT=ind, rhs=[p*h | p]).
  - Flush: normalize, mean over heads, bias, relu -> PE transpose ->
    AllGather fp16 (Shared output) -> next layer table rebuild with
    DRAM-contiguous [128,row-pitch] writes.
"""
import numpy as np

N = 20000
NCORES = 8
NPC = 2500
NPAD = 2560
NBLK = 20
NTOT = NCORES * NPAD  # 20480
P = 128

LAST_RESULT = None


# ----------------------------------------------------------------- host prep
def _wrap16(idx, ncols):
    n = len(idx)
    w = np.zeros((P, ncols), dtype=np.int16)
    cols = (n + 15) // 16
    assert cols <= ncols
    buf = np.zeros((16, cols), dtype=np.int16)
    buf[np.arange(n) % 16, np.arange(n) // 16] = idx
    for g in range(8):
        w[16 * g:16 * g + 16, :cols] = buf
    return w


def _preprocess(edge_index, x):
    src = np.asarray(edge_index[0], dtype=np.int64)
    dst = np.asarray(edge_index[1], dtype=np.int64)
    # self-loops are handled locally on-device, not via gather

    own_s = src // NPC
    src_p = own_s * NPAD + (src - own_s * NPC)
    own = dst // NPC
    dst_loc = dst - own * NPC

    order = np.lexsort((dst_loc, own))
    src_p, dst_loc, own = src_p[order], dst_loc[order], own[order]
    blk = dst_loc // P
    counts = np.zeros((NCORES, NBLK), dtype=np.int64)
    for c in range(NCORES):
        for b in range(NBLK):
            counts[c, b] = np.sum((own == c) & (blk == b))
    T = np.maximum(1, np.ceil(counts.max(axis=0) / P).astype(np.int64))
    Ttot = int(T.sum())

    # padded input x (node row n lives at padded id own*NPAD + local)
    x = np.asarray(x, dtype=np.float32)
    xpad = np.zeros((NTOT, 128), dtype=np.float32)
    for c in range(NCORES):
        xpad[c * NPAD:c * NPAD + NPC] = x[c * NPC:(c + 1) * NPC]

    wrap_src = np.zeros((NCORES, P, Ttot * 8), dtype=np.int16)
    dstloc16 = np.full((NCORES, P, Ttot), -1.0, dtype=np.float16)
    import ml_dtypes
    indT = np.zeros((NCORES, P, Ttot * P), dtype=ml_dtypes.float8_e4m3)
    xeT = np.zeros((NCORES, P, Ttot * P), dtype=np.float16)
    off8 = np.zeros(NBLK + 1, dtype=np.int64)
    offT = np.zeros(NBLK + 1, dtype=np.int64)
    for b in range(NBLK):
        off8[b + 1] = off8[b] + T[b] * 8
        offT[b + 1] = offT[b] + T[b]
    prow = np.arange(P, dtype=np.float32)[:, None]
    for c in range(NCORES):
        m_c = own == c
        for b in range(NBLK):
            m = m_c & (blk == b)
            cnt = int(counts[c, b])
            nb = int(T[b]) * P
            isrc = np.zeros(nb, dtype=np.int64)
            isrc[:cnt] = src_p[m]
            dl = np.full(nb, -1.0, dtype=np.float32)
            dl[:cnt] = dst_loc[m] - b * P
            wrap_src[c, :, off8[b]:off8[b + 1]] = _wrap16(isrc, int(T[b]) * 8)
            dstloc16[c, :, offT[b]:offT[b + 1]] = (
                dl.reshape(int(T[b]), P).T.astype(np.float16))
            indT[c, :, offT[b] * P:offT[b + 1] * P] = (
                prow == dl[None, :]).astype(ml_dtypes.float8_e4m3)
            xeT[c, :, offT[b] * P:offT[b + 1] * P] = (
                xpad[isrc].T.astype(np.float16))
    return T, off8, offT, wrap_src, dstloc16, indT, xeT


# ------------------------------------------------------------- build program
def _build(T, off8, offT, do_compile=True):
    from concourse import bass, bacc, mybir, tile

    f16 = mybir.dt.float16
    f32 = mybir.dt.float32
    i16 = mybir.dt.int16
    f8 = mybir.dt.float8e4
    AF = mybir.ActivationFunctionType
    OP = mybir.AluOpType

    Ttot = int(T.sum())
    NW = Ttot * 8
    Tmax = int(T.max())
    NVALID_LAST = NPC - (NBLK - 1) * P  # 68

    nc = bacc.Bacc("TRN2", target_bir_lowering=False, debug=False,
                   num_devices=NCORES)

    # inputs
    xlocT = nc.dram_tensor("xlocT", [P, NPAD], f16, kind="ExternalInput")
    iwsrc = nc.dram_tensor("iwsrc", [P, NW], i16, kind="ExternalInput")
    dloc = nc.dram_tensor("dloc", [P, Ttot], f16, kind="ExternalInput")
    indT_d = nc.dram_tensor("indT", [P, Ttot * P], f8, kind="ExternalInput")
    xeT_d = nc.dram_tensor("xeT", [P, Ttot * P], f16, kind="ExternalInput")
    iotabig = nc.dram_tensor("iotabig", [P, Tmax * P], f16,
                             kind="ExternalInput")
    c100 = nc.dram_tensor("c100", [P, 32], f32, kind="ExternalInput")
    c1em8 = nc.dram_tensor("c1em8", [P, 32], f32, kind="ExternalInput")
    ident16 = nc.dram_tensor("ident16", [P, P], f16, kind="ExternalInput")
    identf = nc.dram_tensor("identf", [P, P], f32, kind="ExternalInput")
    w1c = nc.dram_tensor("w1c", [128, 260], f16, kind="ExternalInput")
    w2c = nc.dram_tensor("w2c", [64, 256], f16, kind="ExternalInput")
    asf2 = nc.dram_tensor("asf2", [P, 256], f16, kind="ExternalInput")
    asfr2 = nc.dram_tensor("asfr2", [P, Tmax * 256], f16,
                           kind="ExternalInput")
    w3c = nc.dram_tensor("w3c", [64, 128], f16, kind="ExternalInput")
    wad1 = nc.dram_tensor("wad1", [128, 4], f16, kind="ExternalInput")
    wad2 = nc.dram_tensor("wad2", [64, 4], f16, kind="ExternalInput")
    wad3 = nc.dram_tensor("wad3", [64, 1], f16, kind="ExternalInput")
    b1r = nc.dram_tensor("b1r", [P, 64], f32, kind="ExternalInput")
    b2r = nc.dram_tensor("b2r", [P, 64], f32, kind="ExternalInput")
    b3r = nc.dram_tensor("b3r", [P, 32], f32, kind="ExternalInput")
    bmr = nc.dram_tensor("bmr", [P, 32], f32, kind="ExternalInput")
    bvr = nc.dram_tensor("bvr", [P, 32], f32, kind="ExternalInput")
    wm = nc.dram_tensor("wm", [32, 32], f32, kind="ExternalInput")
    wv = nc.dram_tensor("wv", [32, 32], f32, kind="ExternalInput")

    # outputs
    z_out = nc.dram_tensor("z", [NPC, 32], f32, kind="ExternalOutput")
    zm_out = nc.dram_tensor("zmean", [NPC, 32], f32, kind="ExternalOutput")
    zv_out = nc.dram_tensor("zvar", [NPC, 32], f32, kind="ExternalOutput")

    with tile.TileContext(nc) as tc:
        with (
            tc.tile_pool(name="const", bufs=1) as cpool,
            tc.tile_pool(name="sb", bufs=3) as sb,
            tc.tile_pool(name="gth", bufs=3) as gth,
            tc.tile_pool(name="blk", bufs=3) as blk,
            tc.tile_pool(name="pxp", bufs=3) as pxp,
            tc.tile_pool(name="pswk", bufs=3, space="PSUM") as pswk,
            tc.tile_pool(name="psad", bufs=2, space="PSUM") as psad,
            tc.tile_pool(name="pssm", bufs=1, space="PSUM") as pssm,
            tc.tile_pool(name="psagg", bufs=2, space="PSUM") as psagg,
            tc.tile_pool(name="dram", bufs=1, space="DRAM") as dram,
        ):
            # per-layer node tables: local slice built during the previous
            # layer's flush, then one AllGather -> full Shared table
            tabloc2 = dram.tile([NPAD, 256], f16)
            tabloc3 = dram.tile([NPAD, 128], f16)
            tab2 = dram.tile([NCORES, NPAD, 256], f16,
                             addr_space="Shared")
            tab3 = dram.tile([NCORES, NPAD, 128], f16, addr_space="Shared")
            adloc2 = dram.tile([NPAD, 4], f16)
            adloc3 = dram.tile([NPAD, 1], f16)

            def ld(shape, dt, src):
                t = cpool.tile(shape, dt, tag="c_" + src.name)
                nc.sync.dma_start(out=t[:], in_=src[:, :])
                return t

            id16_sb = ld([P, P], f16, ident16)
            idf_sb = ld([P, P], f32, identf)
            w1c_sb = ld([128, 260], f16, w1c)
            w2c_sb = ld([64, 256], f16, w2c)
            asf2_sb = ld([P, 256], f16, asf2)
            asfr2_sb = ld([P, Tmax * 256], f16, asfr2)
            w3c_sb = ld([64, 128], f16, w3c)
            wad1_sb = ld([128, 4], f16, wad1)
            wad2_sb = ld([64, 4], f16, wad2)
            wad3_sb = ld([64, 1], f16, wad3)
            b1r_sb = ld([P, 64], f32, b1r)
            b2r_sb = ld([P, 64], f32, b2r)
            b3r_sb = ld([P, 32], f32, b3r)
            bmr_sb = ld([P, 32], f32, bmr)
            bvr_sb = ld([P, 32], f32, bvr)
            wm_sb = ld([32, 32], f32, wm)
            wv_sb = ld([32, 32], f32, wv)
            iwsrc_sb = ld([P, NW], i16, iwsrc)
            dloc_sb = ld([P, Ttot], f16, dloc)
            iotabig_sb = ld([P, Tmax * P], f16, iotabig)
            c100_sb = ld([P, 32], f32, c100)
            c1em8_sb = ld([P, 32], f32, c1em8)
            xloc_sb = ld([P, NPAD], f16, xlocT)

            # layer-1 alpha_dst for all blocks, computed once at the head
            adtab1 = cpool.tile([P, NBLK * 4], f16, tag="adtab1")
            for b in range(NBLK):
                pad1 = pssm.tile([P, 4], f32, space="PSUM", tag="sm")
                nc.tensor.matmul(out=pad1[:],
                                 lhsT=xloc_sb[:, b * P:(b + 1) * P],
                                 rhs=wad1_sb[:, :], start=True, stop=True)
                nc.scalar.activation(adtab1[:, b * 4:(b + 1) * 4], pad1[:],
                                     AF.Copy)

            # -------- shared per-block helpers ---------------------------
            def load_indicators(b, Tb):
                # ind[p,t,q] = (dloc[p, offT+t] == q) : edge (t,p) -> dst q
                ind = blk.tile([P, Tb, P], f8, tag="ind")
                nc.vector.tensor_tensor(
                    out=ind[:],
                    in0=dloc_sb[:, int(offT[b]):int(offT[b]) + Tb, None]
                    .to_broadcast([P, Tb, P]),
                    in1=iotabig_sb[:, :Tb * P]
                    .rearrange("p (t q) -> p t q", t=Tb),
                    op=OP.is_equal)
                indT = blk.tile([P, Tb, P], f8, tag="indT")
                nc.sync.dma_start(
                    out=indT[:],
                    in_=indT_d[:, int(offT[b]) * P:int(offT[b + 1]) * P]
                    .rearrange("p (t q) -> p t q", t=Tb))
                return ind, indT

            def leaky_exp_into(pex, pes, Tb, H):
                # pes: [P,Tb,H] f32 pre-activation; writes
                # exp(leaky_relu(pes)) into pex[:, :, HC:HC+H] fp16
                es = sb.tile([P, Tb, H], f32, tag="es")
                nc.vector.tensor_scalar_mul(out=es[:], in0=pes[:],
                                            scalar1=0.2)
                nc.vector.tensor_tensor(out=es[:], in0=es[:], in1=pes[:],
                                        op=OP.max)
                HC = pex.shape[2] - H
                nc.scalar.activation(pex[:, :, HC:HC + H], es[:], AF.Exp)
                return es

            def self_term(as_self, ad_self, h_self, H, C):
                # exp(leaky(as+ad)) * h for the block's own nodes (the
                # self-loop edge), returned as an agg-matmul rhs tile
                HC = H * C
                ess = sb.tile([P, H], f32, tag="ess")
                nc.vector.tensor_tensor(out=ess[:], in0=as_self,
                                        in1=ad_self, op=OP.add)
                es2 = sb.tile([P, H], f32, tag="ess2")
                nc.vector.tensor_scalar_mul(out=es2[:], in0=ess[:],
                                            scalar1=0.2)
                nc.vector.tensor_tensor(out=ess[:], in0=ess[:], in1=es2[:],
                                        op=OP.max)
                pxs = pxp.tile([P, HC + H], f16, tag="pxs")
                nc.scalar.activation(pxs[:, HC:HC + H], ess[:], AF.Exp)
                nc.vector.tensor_tensor(
                    out=pxs[:, 0:HC].rearrange("p (h c) -> p h c", h=H),
                    in0=h_self.rearrange("p (h c) -> p h c", h=H),
                    in1=pxs[:, HC:HC + H, None].to_broadcast([P, H, C]),
                    op=OP.mult)
                return pxs

            # -------- layer 1 stage: streamed per-edge xeT, no gather ----
            def stage1(b):
                H, C = 4, 64
                HC = H * C
                Tb = int(T[b])
                xet = gth.tile([P, Tb * P], f16, tag="xet")
                nc.sync.dma_start(
                    out=xet[:],
                    in_=xeT_d[:, int(offT[b]) * P:int(offT[b + 1]) * P])
                ind, indT = load_indicators(b, Tb)
                adb = adtab1[:, b * 4:(b + 1) * 4]

                # es pre-activation: as1[src_e] + ad1[dst_e]
                pes = psad.tile([P, Tb, H], f32, space="PSUM", tag="pes")
                for t in range(Tb):
                    nc.tensor.matmul(
                        out=pes[:, t, :], lhsT=xet[:, t * P:(t + 1) * P],
                        rhs=w1c_sb[:, HC:HC + H], start=True, stop=False)
                    nc.tensor.matmul(
                        out=pes[:, t, :], lhsT=indT[:, t, :],
                        rhs=adb, start=False, stop=True)
                pex = pxp.tile([P, Tb, HC + H], f16, tag="pex")
                leaky_exp_into(pex, pes, Tb, H)

                # h per edge + p*h, tile by tile
                for t in range(Tb):
                    ph = pswk.tile([P, HC], f32, space="PSUM",
                                   padded_shape=[P, 384], tag="wk")
                    nc.tensor.matmul(
                        out=ph[:], lhsT=xet[:, t * P:(t + 1) * P],
                        rhs=w1c_sb[:, 0:HC], start=True, stop=True)
                    nc.vector.tensor_tensor(
                        out=pex[:, t, 0:HC]
                        .rearrange("p (h c) -> p h c", h=H),
                        in0=ph[:].rearrange("p (h c) -> p h c", h=H),
                        in1=pex[:, t, HC:HC + H, None]
                        .to_broadcast([P, H, C]),
                        op=OP.mult)
                # self-loop term: h1|as1 of the block's own nodes
                phs = pswk.tile([P, HC + H], f32, space="PSUM",
                                padded_shape=[P, 384], tag="wk")
                nc.tensor.matmul(out=phs[:],
                                 lhsT=xloc_sb[:, b * P:(b + 1) * P],
                                 rhs=w1c_sb[:, 0:HC + H],
                                 start=True, stop=True)
                pxs = self_term(phs[:, HC:HC + H], adb, phs[:, 0:HC], H, C)
                return ind, pex, pxs

            # -------- layers 2,3 stage: gather-based ---------------------
            def stage23(tab, adloc, tabloc, elem, H, C, b, asf=None):
                HC = H * C
                Tb = int(T[b])
                nidx = Tb * P
                g = gth.tile([P, Tb, elem], f16, tag="g")
                nc.gpsimd.dma_gather(
                    out_ap=g[:], in_ap=tab[:].rearrange("c n k -> (c n) k"),
                    idxs_ap=iwsrc_sb[:, int(off8[b]):int(off8[b]) + Tb * 8],
                    num_idxs=nidx, num_idxs_reg=nidx, elem_size=elem,
                    elem_step=elem, single_packet=False)
                ind, indT = load_indicators(b, Tb)
                adb = sb.tile([P, H], f16, tag="adb")
                nc.sync.dma_start(out=adb[:],
                                  in_=adloc[b * P:(b + 1) * P, :])

                pad_all = psad.tile([P, Tb, H], f32, space="PSUM", tag="pes")
                for t in range(Tb):
                    nc.tensor.matmul(out=pad_all[:, t, :],
                                     lhsT=indT[:, t, :],
                                     rhs=adb[:], start=True, stop=True)
                pes = sb.tile([P, Tb, H], f32, tag="pess")
                if asf is None:
                    nc.vector.tensor_tensor(out=pes[:],
                                            in0=g[:, :, HC:HC + H],
                                            in1=pad_all[:], op=OP.add)
                else:
                    # alpha_src per edge = sum_c h[e,h,c] * a_s[h,c]
                    tmp = pxp.tile([P, Tb, HC], f16, tag="ast")
                    nc.vector.tensor_tensor(
                        out=tmp[:], in0=g[:, :, 0:HC],
                        in1=asfr2_sb[:, :Tb * HC]
                        .rearrange("p (t k) -> p t k", t=Tb),
                        op=OP.mult)
                    th2 = sb.tile([P, Tb, H, C // 2], f16, tag="th2")
                    tv = tmp[:].rearrange("p t (h c) -> p t h c", h=H)
                    nc.vector.tensor_tensor(out=th2[:], in0=tv[:, :, :, 0:C // 2],
                                            in1=tv[:, :, :, C // 2:C],
                                            op=OP.add)
                    asp = sb.tile([P, Tb, H], f32, tag="asp")
                    nc.vector.tensor_reduce(
                        out=asp[:], in_=th2[:],
                        axis=mybir.AxisListType.X, op=OP.add)
                    nc.vector.tensor_tensor(out=pes[:], in0=asp[:],
                                            in1=pad_all[:], op=OP.add)
                tsf = sb.tile([P, elem], f16, tag="tself")
                nc.sync.dma_start(out=tsf[:],
                                  in_=tabloc[b * P:(b + 1) * P, :])
                if asf is None:
                    as_self = tsf[:, HC:HC + H]
                else:
                    tmps = sb.tile([P, HC], f16, tag="tmps")
                    nc.vector.tensor_tensor(out=tmps[:], in0=tsf[:, 0:HC],
                                            in1=asf[:, :HC], op=OP.mult)
                    asps = sb.tile([P, H], f32, tag="asps")
                    nc.vector.tensor_reduce(
                        out=asps[:],
                        in_=tmps[:].rearrange("p (h c) -> p h c", h=H),
                        axis=mybir.AxisListType.X, op=OP.add)
                    as_self = asps[:]
                pxs = self_term(as_self, adb[:], tsf[:, 0:HC], H, C)
                pex = pxp.tile([P, Tb, HC + H], f16, tag="pex")
                es = leaky_exp_into(pex, pes, Tb, H)
                if asf is not None:
                    pexf = pxp.tile([P, Tb, HC], f16, tag="ast")
                    nc.scalar.activation(
                        pexf[:].rearrange("p t (h c) -> p t h c", h=H),
                        es[:, :, :, None].to_broadcast([P, Tb, H, C]),
                        AF.Exp)
                    nc.vector.tensor_tensor(out=pex[:, :, 0:HC],
                                            in0=g[:, :, 0:HC], in1=pexf[:],
                                            op=OP.mult)
                else:
                    nc.vector.tensor_tensor(
                        out=pex[:, :, 0:HC]
                        .rearrange("p t (h c) -> p t h c", h=H),
                        in0=g[:, :, 0:HC]
                        .rearrange("p t (h c) -> p t h c", h=H),
                        in1=pex[:, :, HC:HC + H, None]
                        .to_broadcast([P, Tb, H, C]),
                        op=OP.mult)
                return ind, pex, pxs

            def agg_flush(b, state, ncols, flush):
                ind, pex, pxs = state
                Tb = int(T[b])
                pa = psagg.tile([P, ncols], f32, space="PSUM", tag="agg")
                for t in range(Tb):
                    nc.tensor.matmul(
                        out=pa[:], lhsT=ind[:, t, :], rhs=pex[:, t, :],
                        start=(t == 0), stop=False)
                nc.tensor.matmul(out=pa[:], lhsT=id16_sb[:],
                                 rhs=pxs[:, :ncols], start=False, stop=True)
                flush(b, pa)

            def edge_layer(stage, ncols, flush):
                # software pipeline: block b's gather/indicator/exp chain is
                # issued before block b-1's aggregation+flush on every engine
                prev = stage(0)
                for b in range(1, NBLK):
                    cur = stage(b)
                    agg_flush(b - 1, prev, ncols, flush)
                    prev = cur
                agg_flush(NBLK - 1, prev, ncols, flush)

            # -------- flush -----------------------------------------------
            def write_adloc(xt_sb_blk, wad_sb, in_c, H, adloc, b):
                pad = pssm.tile([P, 4], f32, space="PSUM", tag="sm")
                nc.tensor.matmul(out=pad[:, :H], lhsT=xt_sb_blk,
                                 rhs=wad_sb[:in_c, :H], start=True, stop=True)
                a16 = sb.tile([P, H], f16, tag="a16")
                nc.scalar.activation(a16[:], pad[:, :H], AF.Copy)
                nc.scalar.dma_start(out=adloc[b * P:(b + 1) * P, :],
                                    in_=a16[:])

            def flush_12(b, pa, brep_sb, wcn_sb, pitch, tabloc, wadn_sb,
                         adlocn, Hn):
                H, C = 4, 64
                HC = H * C
                inv = sb.tile([P, H], f32, tag="inv")
                nc.vector.tensor_scalar_add(out=inv[:], in0=pa[:, HC:HC + H],
                                            scalar1=1e-16)
                nc.vector.reciprocal(out=inv[:], in_=inv[:])
                nc.vector.tensor_scalar_mul(out=inv[:], in0=inv[:],
                                            scalar1=1.0 / H)
                nrm = sb.tile([P, HC], f32, tag="nrm")
                nc.vector.tensor_tensor(
                    out=nrm[:].rearrange("p (h c) -> p h c", h=H),
                    in0=pa[:, 0:HC].rearrange("p (h c) -> p h c", h=H),
                    in1=inv[:, :, None].to_broadcast([P, H, C]),
                    op=OP.mult)
                m = sb.tile([P, C], f32, tag="mean")
                nc.vector.tensor_reduce(
                    out=m[:], in_=nrm[:].rearrange("p (h c) -> p c h", h=H),
                    axis=mybir.AxisListType.X, op=OP.add)
                nc.vector.tensor_add(out=m[:], in0=m[:], in1=brep_sb[:, :C])
                x16 = sb.tile([P, C], f16, tag="x16")
                nc.vector.tensor_scalar(out=x16[:], in0=m[:], scalar1=0.0,
                                        scalar2=None, op0=OP.max)
                pt = pssm.tile([C, P], f16, space="PSUM", tag="sm")
                nc.tensor.transpose(out=pt[:], in_=x16[:], identity=id16_sb[:])
                xt = sb.tile([C, P], f16, tag="xt")
                nc.scalar.activation(xt[:], pt[:], AF.Copy)
                # next-layer node-table rows for this core's block
                prt = pswk.tile([P, pitch], f32, space="PSUM",
                                padded_shape=[P, 384], tag="wk")
                nc.tensor.matmul(out=prt[:], lhsT=xt[:],
                                 rhs=wcn_sb[:C, :pitch], start=True, stop=True)
                t16 = sb.tile([P, pitch], f16, tag="trow")
                nc.scalar.activation(t16[:], prt[:], AF.Copy)
                nc.scalar.dma_start(out=tabloc[b * P:(b + 1) * P, :],
                                    in_=t16[:])
                write_adloc(xt[:], wadn_sb, C, Hn, adlocn, b)

            def flush_3(b, pa):
                nvalid = NVALID_LAST if b == NBLK - 1 else P
                inv = sb.tile([P, 1], f32, tag="inv")
                nc.vector.tensor_scalar_add(out=inv[:], in0=pa[:, 32:33],
                                            scalar1=1e-16)
                nc.vector.reciprocal(out=inv[:], in_=inv[:])
                z = sb.tile([P, 32], f32, tag="zf")
                nc.vector.tensor_scalar_mul(out=z[:], in0=pa[:, 0:32],
                                            scalar1=inv[:])
                nc.vector.tensor_add(out=z[:], in0=z[:], in1=b3r_sb[:])
                nc.sync.dma_start(out=z_out[b * P:b * P + nvalid, :],
                                  in_=z[:nvalid, :])
                zt_ps = pssm.tile([32, P], f32, space="PSUM", tag="sm")
                nc.tensor.transpose(out=zt_ps[:], in_=z[:, :32],
                                    identity=idf_sb[:])
                zt = sb.tile([32, P], f32, tag="zt")
                nc.vector.tensor_copy(out=zt[:], in_=zt_ps[:])
                pm = pssm.tile([P, 32], f32, space="PSUM", tag="sm")
                nc.tensor.matmul(out=pm[:], lhsT=zt[:], rhs=wm_sb[:],
                                 start=True, stop=True)
                zm = sb.tile([P, 32], f32, tag="zm")
                nc.vector.tensor_add(out=zm[:], in0=pm[:], in1=bmr_sb[:])
                nc.sync.dma_start(out=zm_out[b * P:b * P + nvalid, :],
                                  in_=zm[:nvalid, :])
                pv = pssm.tile([P, 32], f32, space="PSUM", tag="sm")
                nc.tensor.matmul(out=pv[:], lhsT=zt[:], rhs=wv_sb[:],
                                 start=True, stop=True)
                zv = sb.tile([P, 32], f32, tag="zv")
                nc.vector.tensor_add(out=zv[:], in0=pv[:], in1=bvr_sb[:])
                nc.scalar.activation(zv[:], zv[:], AF.Exp)
                nc.vector.tensor_tensor(out=zv[:], in0=zv[:], in1=c100_sb[:],
                                        op=OP.min)
                nc.vector.tensor_tensor(out=zv[:], in0=zv[:], in1=c1em8_sb[:],
                                        op=OP.max)
                nc.sync.dma_start(out=zv_out[b * P:b * P + nvalid, :],
                                  in_=zv[:nvalid, :])

            # ================ the program ==================================
            edge_layer(stage1, 260,
                       lambda b, pa: flush_12(b, pa, b1r_sb, w2c_sb, 256,
                                              tabloc2, wad2_sb, adloc2, 4))
            nc.gpsimd.collective_compute(
                "AllGather", mybir.AluOpType.bypass,
                replica_groups=[list(range(NCORES))],
                ins=[tabloc2[:]], outs=[tab2[:]])
            edge_layer(lambda b: stage23(tab2, adloc2, tabloc2, 256, 4, 64,
                                         b, asf2_sb), 260,
                       lambda b, pa: flush_12(b, pa, b2r_sb, w3c_sb, 128,
                                              tabloc3, wad3_sb, adloc3, 1))
            nc.gpsimd.collective_compute(
                "AllGather", mybir.AluOpType.bypass,
                replica_groups=[list(range(NCORES))],
                ins=[tabloc3[:]], outs=[tab3[:]])
            edge_layer(lambda b: stage23(tab3, adloc3, tabloc3, 128, 1, 32,
                                         b), 33,
                       flush_3)

    if do_compile:
        nc.compile()
    return nc


def _make_in_maps(x, params, wrap_src, dstloc16, indT, xeT, Tmax):
    x = np.asarray(x, dtype=np.float32)

    def comb(W, a_s, pitch):
        W = np.asarray(W, np.float32)
        a_s = np.asarray(a_s, np.float32)
        heads, c = a_s.shape
        Wr = W.reshape(W.shape[0], heads, c)
        was = np.einsum('ihc,hc->ih', Wr, a_s)
        out = np.zeros((W.shape[0], pitch), dtype=np.float16)
        out[:, :W.shape[1]] = W.astype(np.float16)
        out[:, W.shape[1]:W.shape[1] + heads] = was.astype(np.float16)
        return out

    def wadf(W, a_d):
        W = np.asarray(W, np.float32)
        a_d = np.asarray(a_d, np.float32)
        heads, c = a_d.shape
        Wr = W.reshape(W.shape[0], heads, c)
        return np.einsum('ihc,hc->ih', Wr, a_d).astype(np.float16)

    def rep(v, n=P):
        v = np.asarray(v, np.float32).reshape(1, -1)
        return np.repeat(v, n, axis=0).astype(np.float32)

    common = dict(
        iotabig=np.tile(np.arange(P, dtype=np.float16), (P, Tmax)),
        c100=np.full((P, 32), 100.0, dtype=np.float32),
        c1em8=np.full((P, 32), 1e-8, dtype=np.float32),
        ident16=np.eye(P, dtype=np.float16),
        identf=np.eye(P, dtype=np.float32),
        w1c=comb(params['W1'], params['as1'], 260),
        w2c=np.asarray(params['W2'], np.float32).astype(np.float16),
        asf2=np.tile(np.asarray(params['as2'], np.float32)
                     .reshape(1, -1).astype(np.float16), (P, 1)),
        asfr2=np.tile(np.asarray(params['as2'], np.float32)
                      .reshape(1, -1).astype(np.float16), (P, Tmax)),
        w3c=comb(params['W3'], params['as3'], 128),
        wad1=wadf(params['W1'], params['ad1']),
        wad2=wadf(params['W2'], params['ad2']),
        wad3=wadf(params['W3'], params['ad3']),
        b1r=rep(params['b1']), b2r=rep(params['b2']), b3r=rep(params['b3']),
        bmr=rep(params['bm']), bvr=rep(params['bv']),
        wm=np.asarray(params['Wm'], np.float32),
        wv=np.asarray(params['Wv'], np.float32),
    )
    in_maps = []
    for c in range(NCORES):
        xs = x[c * NPC:(c + 1) * NPC]
        xlocT = np.zeros((P, NPAD), dtype=np.float16)
        xlocT[:, :NPC] = xs.T.astype(np.float16)
        m = dict(common)
        m.update(iwsrc=wrap_src[c], dloc=dstloc16[c], indT=indT[c],
                 xeT=xeT[c], xlocT=xlocT)
        in_maps.append(m)
    return in_maps


# ------------------------------------------------------------------ driver
def kernel(x, edge_index, W1, as1, ad1, b1, W2, as2, ad2, b2,
           W3, as3, ad3, b3, Wm, bm, Wv, bv):
    global LAST_RESULT
    import os
    from concourse.bass_utils import run_bass_kernel_spmd

    T, off8, offT, wrap_src, dstloc16, indT, xeT = _preprocess(
        np.asarray(edge_index), x)
    params = dict(W1=W1, as1=as1, ad1=ad1, b1=b1, W2=W2, as2=as2, ad2=ad2,
                  b2=b2, W3=W3, as3=as3, ad3=ad3, b3=b3, Wm=Wm, bm=bm,
                  Wv=Wv, bv=bv)
    in_maps = _make_in_maps(x, params, wrap_src, dstloc16, indT, xeT,
                            int(T.max()))

    nc = _build(T, off8, offT)
    res = run_bass_kernel_spmd(
        nc, in_maps, core_ids=list(range(NCORES)),
        trace=os.environ.get("BASS_TRACE", "") not in ("", "0"))
    LAST_RESULT = res

    z = np.concatenate([res.results[c]["z"] for c in range(NCORES)], axis=0)
    zm = np.concatenate([res.results[c]["zmean"] for c in range(NCORES)],
                        axis=0)
    zv = np.concatenate([res.results[c]["zvar"] for c in range(NCORES)],
                        axis=0)
    return zm, zv, z
